# revision 69
# baseline (speedup 1.0000x reference)
"""Trainium2 Bass kernel for nn_Attention_5299989643989.

GQA attention forward (B=2, T=2048, C=1024, 16 q heads / 4 kv heads, D=64)
with value-embedding gating, rotary embedding, qk rms-norm, causal softmax.

Sharding: 8 cores = batch (2) x kv-head-group (4).  Each core computes its
4 q heads / 1 kv head end-to-end plus the Wo row-shard partial output; the
host sums the 4 partials per batch (the Wo all-reduce, done at unshard).

Per-core structure (bf16 data paths, fp32 PSUM accumulate; ~141us/core in
the TRN2 cost-model timeline at rel err 5.7e-3, vs 175us/2.6e-4 for the
all-fp32r ancestor). Emission order is tuned so the score/exp stream owns
the in-order engine-queue heads: the next group's projections and the
previous group's normalize/Wo are emitted INSIDE phase2 via a per-head
hook (deferral placement is performance-sensitive).
  phase1a: per 128-token chunk one jammed projection matmul
           [q(256)|k(64)|v(64)|gate(1)|pad] over bf16 x/W accumulated in
           PSUM and copied to a bf16 SBUF group tile; per 4-chunk group:
           rope over all 20 head instances via 4D strided bf16 views (2x
           DVE), rms rstd via bit-trick + ONE Newton step on DVE (bf16
           squares), per-chunk normalize multiplies so each chunk's
           transposes unblock early, one batched sigmoid-gate Tanh,
           ve-gating on GPSIMD.
  phase1b: paired 2-head bf16 PE transposes (1 cycle/row) into qT
           [128,2,T] and row-duplicated kT2 [128,T] (matmul requires equal
           stationary/moving partition bases).
  phase2:  per (head, 512-query block): scoresT tiles [128 keys, <=512
           live queries] = kT^T q; the causal mask is a -3e4 additive
           mask-matmul (trin^T x identity_bf16) accumulated into the
           diagonal PSUM band so exp underflows masked lanes to exactly 0
           (no Pool multiply, no extra engine hop); exp on ACT (the one
           saturated engine mid-run) with the folded 1.2*1.2/sqrt(64)
           scale, bf16 out; yT [65,512] += v_aug^T expT with a ones column
           producing denominators for free; 4-slot PSUM score pipeline
           with depth-4 AV stagger, 9 ex buffers.
  norm3:   denominator reciprocals on DVE, f32r rounding copy (Pool
           mid-run / ACT in the drain tail), PE outer-product broadcast,
           yT scaling, row-sharded bf16 Wo, per-chunk [128,1024] staging
           tile with PSUM readouts alternating DVE/ACT in the tail, one
           DMA per token chunk.
  Software pipeline at emission: group bi+1's projections and DVE chain
  are emitted before phase2(bi); normalize/Wo of bi-1 fill the PE while
  bi's transposes wait on the DVE chain; the last group's pair-0
  normalize is emitted mid-phase2 so only pair 1 + Wo remain in the tail.
Host side ships bf16 inputs (x^T, W-jam, cos/sin, 3*ve, Wo^T, mask) and
upcasts the bf16 per-core partials while summing the Wo row-shards.
"""

import numpy as np

import concourse.bacc as bacc
import concourse.bass as bass
import concourse.tile as tile
from concourse import mybir
from concourse.masks import make_identity

f32 = mybir.dt.float32
f32r = mybir.dt.float32r
bf16 = mybir.dt.bfloat16
AF = mybir.ActivationFunctionType

B, T, C = 2, 2048, 1024
N_HEAD, N_KV_HEAD, D = 16, 4, 64
HQ = N_HEAD // N_KV_HEAD  # q heads per core = 4
P = 128
NT = T // P       # 16 token chunks
KC = C // P       # 8 contraction chunks
IB = 512          # query block
NBI = T // IB     # 4 query blocks
GRP = IB // P     # 4 token chunks per query block
SC = 1.2 * 1.2 / 8.0  # folded qk scale: rms 1.2 factors * 1/sqrt(64)
H32 = D // 2


def build_program():
    nc = bacc.Bacc("TRN2", target_bir_lowering=False, debug=False, num_devices=8)

    xT = nc.dram_tensor("xT", [C, T], bf16, kind="ExternalInput")
    wr = nc.dram_tensor("wr", [C, 386], bf16, kind="ExternalInput")
    # cos/sin/ve3 pre-swizzled on the host into the SBUF-resident layout so
    # the DMA moves long contiguous rows (512B+ descriptors, full bandwidth)
    cosd = nc.dram_tensor("cosd", [P, NT * 32], bf16, kind="ExternalInput")
    sind = nc.dram_tensor("sind", [P, NT * 32], bf16, kind="ExternalInput")
    ve3 = nc.dram_tensor("ve3", [P, NT * D], bf16, kind="ExternalInput")
    woT = nc.dram_tensor("woT", [2 * P, C], bf16, kind="ExternalInput")
    trind = nc.dram_tensor("trind", [P, P], bf16, kind="ExternalInput")
    out = nc.dram_tensor("out", [T, C], bf16, kind="ExternalOutput")

    with tile.TileContext(nc) as tc:
        with (
            tc.tile_pool(name="consts", bufs=1) as consts,
            tc.tile_pool(name="resid", bufs=1) as resid,
            tc.tile_pool(name="xload", bufs=2) as xload,
            tc.tile_pool(name="rot", bufs=2) as rot,
            tc.tile_pool(name="small", bufs=4) as small,
            tc.tile_pool(name="exps", bufs=9) as exps,
            tc.tile_pool(name="ynp", bufs=5) as ynp,
            tc.tile_pool(name="outsb", bufs=4) as outsb,
            tc.tile_pool(name="psmm", bufs=2, space="PSUM") as psmm,
            tc.tile_pool(name="pssc", bufs=4, space="PSUM") as pssc,
            tc.tile_pool(name="psy", bufs=2, space="PSUM") as psy,
        ):
            # ---- resident loads ----
            # The cost-model DMA lane is serial (~0.003 ns/B), so order
            # strictly by need: weights + cos/sin first (small), then x block
            # 0 token-major in 4 pieces so each projection chunk can run as
            # its tokens arrive, then everything else.
            wr_sb = consts.tile([P, KC, 386], bf16)
            nc.sync.dma_start(
                wr_sb[:, 0, :],
                wr[0:P, :],
            )
            nc.sync.dma_start(
                wr_sb[:, 1:KC, :],
                wr[P:C, :].rearrange("(kc p) c -> p kc c", p=P),
            )
            xt0 = xload.tile([P, KC, IB], bf16, name="xt0", tag="xt")
            nc.sync.dma_start(
                xt0[:, 0 : KC // 2, :],
                xT[0 : C // 2, 0:IB].rearrange("(kc p) t -> p kc t", p=P),
            )
            nc.sync.dma_start(
                xt0[:, KC // 2 : KC, :],
                xT[C // 2 : C, 0:IB].rearrange("(kc p) t -> p kc t", p=P),
            )
            cos_sb = consts.tile([P, NT, 32], bf16)
            nc.sync.dma_start(cos_sb[:].rearrange("p n d -> p (n d)"), cosd[:])
            sin_sb = consts.tile([P, NT, 32], bf16)
            nc.sync.dma_start(sin_sb[:].rearrange("p n d -> p (n d)"), sind[:])
            xt1 = xload.tile([P, KC, IB], bf16, name="xt1", tag="xt")
            nc.sync.dma_start(
                xt1[:],
                xT[:, IB : 2 * IB].rearrange("(kc p) t -> p kc t", p=P),
            )
            ve3_sb = consts.tile([P, NT, D], bf16)
            nc.sync.dma_start(ve3_sb[:].rearrange("p n d -> p (n d)"), ve3[:])
            trin_sb = consts.tile([P, P], bf16)
            nc.sync.dma_start(trin_sb[:], trind[:])
            wo1_sb = consts.tile([P, C], bf16)
            nc.sync.dma_start(wo1_sb[:], woT[0:P, :])
            wo2_sb = consts.tile([P, C], bf16)
            nc.sync.dma_start(wo2_sb[:], woT[P : 2 * P, :])
            ident = consts.tile([P, P], f32)
            make_identity(nc, ident[:])
            identb = consts.tile([P, P], bf16)
            nc.vector.tensor_copy(identb[:], ident[:])
            # PE p-state warmup: keep the PE streaming through the initial
            # DMA wait so the first projection matmuls run at full clock
            # (cold PE costs 1.5-3.7x per matmul until 3us of busy ramp)
            warm = psy.tile([P, P], f32, tag="y")
            for _ in range(44):
                nc.tensor.matmul(warm[:], identb[:], identb[:])
            rsq_k = consts.tile([P, 1], mybir.dt.uint32)
            nc.vector.memset(rsq_k[:], 0x5F3759DF)
            zero_sb = consts.tile([P, 1], f32)
            nc.vector.memset(zero_sb[:], 0.0)

            # ---- residents written by the kernel ----
            qT = resid.tile([P, 2, T], bf16)   # [h0|h1] rows, [h2|h3] rows
            kT2 = resid.tile([P, T], bf16)     # kT duplicated in both row halves
            v_aug = resid.tile([P, NT, D + 1], bf16)  # v plus ones column
            nc.vector.memset(v_aug[:, :, D : D + 1], 1.0)
            yT1 = resid.tile([P, T], bf16)        # yT heads 0,1
            yT2 = resid.tile([P, T], bf16)        # yT heads 2,3

            def load_x(bi, q=None):
                xt = xload.tile([P, KC, IB], bf16, name=f"xt{bi}", tag="xt")
                (q or nc.scalar).dma_start(
                    xt[:],
                    xT[:, bi * IB : (bi + 1) * IB]
                    .rearrange("(kc p) t -> p kc t", p=P),
                )
                return xt

            def phase1(bi, xt, halves=1):
                qkr = rot.tile([P, GRP, 320], bf16, tag="qkr", bufs=1)  # roped q|k
                pjg = rot.tile([P, GRP, 386], bf16, tag="pjg")
                tgg = small.tile([P, GRP], f32, tag="tgg")
                for tl in range(GRP):
                    tc_ = bi * GRP + tl
                    pj = psmm.tile([P, 512], f32, tag="mm")
                    for kc in range(KC):
                        nc.tensor.matmul(
                            pj[:, 0:386],
                            xt[:, kc, tl * P : (tl + 1) * P],
                            wr_sb[:, kc, :],
                            start=(kc == 0),
                            stop=(kc == KC - 1),
                        )
                    # ACT for the startup groups (latency-critical chain);
                    # DVE for the mid-run groups where ACT is exp-saturated
                    if bi < 2:
                        nc.scalar.copy(pjg[:, tl, :], pj[:, 0:386])
                    else:
                        nc.vector.tensor_copy(pjg[:, tl, :], pj[:, 0:386])
                # one tanh over all 4 chunks' gate column (already staged in
                # the pjg copy) instead of 4 per-chunk PSUM reads
                nc.scalar.activation(
                    tgg[:], pjg[:, :, 384], AF.Tanh,
                    scale=0.5, bias=zero_sb[:],
                )

                # rope + rms + rstd + normalize over the group, optionally in
                # two chunk-pair halves (shorter DVE chain before the
                # transposes, at the cost of ~16 extra small DVE ops).
                tmp = rot.tile([P, GRP, 160], bf16, tag="tmp", bufs=1)
                sqg = rot.tile([P, GRP, 320], bf16, tag="sqg", bufs=1)
                msg = small.tile([P, GRP * 5], f32, tag="msg")
                rstdg = small.tile([P, GRP * 5], f32, tag="rstdg")
                nwt = small.tile([P, GRP * 5], f32, tag="nwt")
                qkn = rot.tile([P, GRP, 320], bf16, tag="qkn", bufs=2)
                gstep = GRP // halves
                for hf in range(halves):
                    g0, g1_ = gstep * hf, gstep * (hf + 1)
                    f0, f1 = 5 * gstep * hf, 5 * gstep * (hf + 1)
                    nf = f1 - f0
                    qv5 = pjg[:, g0:g1_, 0:320].rearrange(
                        "p g (h d) -> p g h d", d=D
                    )
                    ro5 = qkr[:, g0:g1_, :].rearrange("p g (h d) -> p g h d", d=D)
                    t5 = tmp[:, g0:g1_, :].rearrange("p g (h d) -> p g h d", d=H32)
                    cs = cos_sb[:, bi * GRP + g0 : bi * GRP + g1_, :]
                    sn = sin_sb[:, bi * GRP + g0 : bi * GRP + g1_, :]
                    cos5 = cs.unsqueeze(2).broadcast_to([P, g1_ - g0, 5, H32])
                    sin5 = sn.unsqueeze(2).broadcast_to([P, g1_ - g0, 5, H32])
                    q1 = qv5[:, :, :, 0:H32]
                    q2 = qv5[:, :, :, H32:D]
                    nc.vector.tensor_mul(ro5[:, :, :, 0:H32], q1, cos5)
                    nc.vector.tensor_mul(t5[:], q2, sin5)
                    nc.vector.tensor_add(
                        ro5[:, :, :, 0:H32], ro5[:, :, :, 0:H32], t5[:]
                    )
                    nc.vector.tensor_mul(ro5[:, :, :, H32:D], q2, cos5)
                    nc.vector.tensor_mul(t5[:], q1, sin5)
                    nc.vector.tensor_sub(
                        ro5[:, :, :, H32:D], ro5[:, :, :, H32:D], t5[:]
                    )

                    nc.vector.tensor_mul(
                        sqg[:, g0:g1_, :], qkr[:, g0:g1_, :], qkr[:, g0:g1_, :]
                    )
                    nc.vector.reduce_sum(
                        msg[:, f0:f1],
                        sqg[:, g0:g1_, :].rearrange("p g (h d) -> p (g h) d", d=D),
                        axis=mybir.AxisListType.X,
                    )
                    # m = mean + eps; rstd = m^-1/2 by bit-trick seed + two
                    # Newton iterations, entirely on DVE (no ACT Ln table).
                    nc.vector.tensor_scalar(
                        msg[:, f0:f1], msg[:, f0:f1], 1.0 / D, 1e-6,
                        op0=mybir.AluOpType.mult, op1=mybir.AluOpType.add,
                    )
                    rstdu = rstdg[:, f0:f1].bitcast(mybir.dt.uint32)
                    nc.vector.tensor_scalar(
                        rstdu, msg[:, f0:f1].bitcast(mybir.dt.uint32), 1, None,
                        op0=mybir.AluOpType.logical_shift_right,
                    )
                    nc.vector.tensor_sub(
                        rstdu,
                        rsq_k[:].broadcast_to([P, nf]).bitcast(mybir.dt.uint32),
                        rstdu,
                    )
                    # one Newton step suffices: 3.4% seed error -> ~0.2%,
                    # well inside the bf16 data path's noise floor
                    for _ in range(1):
                        nc.vector.tensor_mul(
                            nwt[:, f0:f1], msg[:, f0:f1], rstdg[:, f0:f1]
                        )
                        nc.vector.tensor_mul(
                            nwt[:, f0:f1], nwt[:, f0:f1], rstdg[:, f0:f1]
                        )
                        nc.vector.tensor_scalar(
                            nwt[:, f0:f1], nwt[:, f0:f1], -0.5, 1.5,
                            op0=mybir.AluOpType.mult, op1=mybir.AluOpType.add,
                        )
                        nc.vector.tensor_mul(
                            rstdg[:, f0:f1], rstdg[:, f0:f1], nwt[:, f0:f1]
                        )
                    # normalize per token chunk so each chunk's transposes
                    # unblock as soon as its multiply lands
                    for g in range(g0, g1_):
                        nc.vector.tensor_mul(
                            qkn[:, g, :].rearrange("p (h d) -> p h d", d=D),
                            qkr[:, g, :].rearrange("p (h d) -> p h d", d=D),
                            rstdg[:, 5 * g : 5 * (g + 1)]
                            .unsqueeze(2)
                            .broadcast_to([P, 5, D]),
                        )
                # gate r = sigmoid(z) = 0.5 + 0.5*tanh(z/2); ve3 is 3*ve.
                # Emitted after the rms chain so the DVE reaches the chain
                # sooner; elementwise v work runs on the idle GPSIMD.
                rgg = small.tile([P, GRP], f32, tag="rgg")
                nc.vector.tensor_scalar(
                    rgg[:], tgg[:], 0.5, 0.5,
                    op0=mybir.AluOpType.mult, op1=mybir.AluOpType.add,
                )
                vtg = small.tile([P, GRP, D], f32, tag="vtg", bufs=1)
                nc.gpsimd.tensor_mul(
                    vtg[:],
                    ve3_sb[:, bi * GRP : (bi + 1) * GRP, :],
                    rgg[:].unsqueeze(2).broadcast_to([P, GRP, D]),
                )
                nc.gpsimd.tensor_add(
                    v_aug[:, bi * GRP : (bi + 1) * GRP, 0:D],
                    pjg[:, :, 320:384],
                    vtg[:],
                )
                qkns[bi] = qkn

            def phase1b(bi):
                qkn = qkns.pop(bi)
                # transposes (two heads per [128,128] transpose)
                tpk = pssc.tile([D, 512], bf16, tag="sc")
                for tl in range(GRP):
                    tc_ = bi * GRP + tl
                    tp = pssc.tile([P, 256], bf16, tag="sc")
                    nc.tensor.transpose(
                        tp[:, 0:P], qkn[:, tl, 0:128], identb[:]
                    )
                    nc.tensor.transpose(
                        tp[:, P : 2 * P], qkn[:, tl, 128:256], identb[:]
                    )
                    nc.tensor.transpose(
                        tpk[:, tl * P : (tl + 1) * P], qkn[:, tl, 256:320], identb[:]
                    )
                    nc.vector.tensor_copy(
                        qT[:, :, tc_ * P : (tc_ + 1) * P],
                        tp[:].rearrange("p (g t) -> p g t", g=2),
                    )
                nc.vector.tensor_copy(kT2[0:D, bi * IB : (bi + 1) * IB], tpk[:])
                nc.vector.tensor_copy(kT2[D:P, bi * IB : (bi + 1) * IB], tpk[:])

            def phase2(bi, after_head=None):
                yns = []
                for h in range(HQ):
                    # y accumulated [query, qc, d | den]: one PSUM bank, four
                    # per-qc accumulation regions.  Only (jt=0, qc=0) starts;
                    # the bank-wide pending-zero makes the other qc's first
                    # write a plain store (skip_group_check for the regions).
                    yp = psy.tile([P, GRP, D + 1], f32, tag="y")
                    njt = GRP * (bi + 1)
                    rr = D * (h % 2)
                    qTh = qT[rr : rr + D, h // 2, :]

                    nfull = GRP * bi + 1  # tiles with lo == 0
                    pending = []

                    def score_mm(spc, jt):
                        dg = jt - GRP * bi
                        lo = max(dg, 0) * P
                        nc.tensor.matmul(
                            spc[:, lo:512],
                            kT2[rr : rr + D, jt * P : (jt + 1) * P],
                            qTh[:, bi * IB + lo : (bi + 1) * IB],
                            start=True,
                            stop=(dg < 0),
                        )
                        if dg >= 0:
                            # additive causal mask: -3e4 above the diagonal so
                            # exp underflows to exactly 0 (no Pool multiply)
                            nc.tensor.matmul(
                                spc[:, lo : lo + P],
                                trin_sb[:],
                                identb[:],
                                start=False,
                                stop=True,
                            )
                        return lo, dg

                    def emit_av(jt, exap, dg):
                        # stationary = exp scores [128 keys, 128 queries],
                        # moving = v_aug [128 keys, 65]: 65-col streams (the
                        # ldweights swap is free) instead of 512-col streams
                        for qc in range(GRP):
                            if dg > qc:
                                continue  # whole qc block above the diagonal
                            nc.tensor.matmul(
                                yp[:, qc, :],
                                exap[:, qc * P : (qc + 1) * P],
                                v_aug[:, jt, :],
                                start=(jt == 0 and qc == 0),
                                stop=(jt == GRP * bi + qc),
                                skip_group_check=True,
                            )

                    def flush(n):
                        while len(pending) > n:
                            emit_av(*pending.pop(0))

                    for jt in range(njt):
                        sp = pssc.tile([P, 512], f32, tag="sc", name="sp")
                        ex = exps.tile([P, 512], bf16, tag="ex", name="ex")
                        lo, dg = score_mm(sp, jt)
                        nc.scalar.activation(
                            ex[:, lo:512], sp[:, lo:512], AF.Exp,
                            scale=SC, bias=zero_sb[:],
                        )
                        pending.append((jt, ex, dg))
                        flush(5)
                    flush(0)
                    # the last head's hook (next group's phase1b) fires
                    # before its normalize tail so the qT/kT2 copies get
                    # ahead of the tail ops in the in-order DVE queue: the
                    # next block's first scores depend on them
                    if after_head is not None:
                        after_head(h)
                        fired_last = True
                    else:
                        fired_last = False
                    # normalize in [query, d] orientation: per-partition
                    # denominator scalars.  The PE transposes into the yT
                    # residents are deferred to the end of the bi block so the
                    # in-order PE queue never parks on a transpose whose yn
                    # input is still deep in the DVE queue (that would starve
                    # the score stream and the exp pipeline behind it).
                    rec = small.tile([P, GRP], f32, tag="rec")
                    nc.vector.reciprocal_approx_fast(rec[:], yp[:, :, D])
                    # hybrid normalize: one DVE bulk copy out of PSUM, then
                    # the per-qc scalar multiplies on the idle Pool engine
                    yn = ynp.tile([P, GRP, D], bf16, tag="yn")
                    if bi == NBI - 1 and h == HQ - 1:
                        # drain tail: direct DVE normalize, shortest chain
                        for qc in range(GRP):
                            nc.vector.tensor_scalar(
                                yn[:, qc, :], yp[:, qc, 0:D], rec[:, qc : qc + 1],
                                None, op0=mybir.AluOpType.mult,
                            )
                    else:
                        # hybrid: one DVE bulk copy out of PSUM, then the
                        # per-qc scalar multiplies on the idle Pool engine
                        yc = ynp.tile([P, GRP, D], bf16, tag="yc")
                        nc.vector.tensor_copy(yc[:], yp[:, :, 0:D])
                        for qc in range(GRP):
                            nc.gpsimd.tensor_scalar(
                                yn[:, qc, :], yc[:, qc, :], rec[:, qc : qc + 1],
                                None, op0=mybir.AluOpType.mult,
                            )
                    yns.append((h, yn))
                    if after_head is not None and not fired_last:
                        after_head(h)
                for h, yn in yns:
                    stg = psy.tile([D, IB], bf16, tag="y")
                    for qc in range(GRP):
                        nc.tensor.transpose(
                            stg[:, qc * P : (qc + 1) * P], yn[:, qc, :], identb[:]
                        )
                    ytp = yT1 if h < 2 else yT2
                    row = D * (h % 2)
                    nc.vector.tensor_copy(
                        ytp[row : row + D, bi * IB : (bi + 1) * IB], stg[:]
                    )

            def norm3w(bi, split_copies=False):
                # in the drain tail the score pool is free: 4 po slots keep
                # the Wo stream, readout copies and out-DMAs fully pipelined
                for tl in range(GRP):
                    tc_ = bi * GRP + tl
                    ob = outsb.tile([P, C], bf16, tag="ob")
                    for cb in range(2):
                        if split_copies:
                            po = pssc.tile([P, 512], f32, tag="sc")
                        else:
                            po = psmm.tile([P, 512], f32, tag="mm")
                        nc.tensor.matmul(
                            po[:],
                            yT1[:, tc_ * P : (tc_ + 1) * P],
                            wo1_sb[:, cb * 512 : (cb + 1) * 512],
                            start=True,
                            stop=False,
                        )
                        nc.tensor.matmul(
                            po[:],
                            yT2[:, tc_ * P : (tc_ + 1) * P],
                            wo2_sb[:, cb * 512 : (cb + 1) * 512],
                            start=False,
                            stop=True,
                        )
                        # in the drain tail ACT is idle: alternate the PSUM
                        # readout between DVE and ACT so po slots recycle 2x
                        # faster
                        if split_copies and cb == 1:
                            nc.scalar.copy(ob[:, cb * 512 : (cb + 1) * 512], po[:])
                        else:
                            nc.vector.tensor_copy(
                                ob[:, cb * 512 : (cb + 1) * 512], po[:]
                            )
                    nc.sync.dma_start(out[tc_ * P : (tc_ + 1) * P, :], ob[:])

            # group-level software pipeline: next group's projections are
            # emitted before the previous group's Wo so the PE has ready work
            # while the per-head normalize chains resolve.
            xts = {0: xt0, 1: xt1}
            qkns = {}
            phase1(0, xts[0], halves=2)
            phase1b(0)

            def hook0(h):
                if h == 0:
                    phase1(1, xts[1])
                    xts[2] = load_x(2)
                elif h == 2:
                    phase1b(1)

            phase2(0, after_head=hook0)
            for bi in range(1, NBI):
                # the next group's projections, transposes and bi-1's
                # Wo/writeout are deferred into phase2 via the per-head hook
                # so they do not sit ahead of the score/exp stream in the
                # in-order queues, and so the transposes finish well before
                # the group boundary
                def hook(h, bi=bi):
                    if h == 0:
                        if bi + 1 < NBI:
                            phase1(bi + 1, xts[bi + 1])
                            if bi + 2 < NBI:
                                xts[bi + 2] = load_x(bi + 2)
                    elif h == 1:
                        norm3w(bi - 1)
                    elif h == 2 and bi + 1 < NBI:
                        phase1b(bi + 1)

                phase2(bi, after_head=hook)
                if bi == NBI - 1:
                    norm3w(bi, split_copies=True)
    nc.compile()
    return nc


def make_core_inputs(x, ve, cos, sin, Wq, Wk, Wv, Wo, Wg):
    """Slice full inputs into the 8 per-core input maps (b-major, then group)."""
    import ml_dtypes

    bf = ml_dtypes.bfloat16
    # device layout [P, NT*32]: row p holds cos[n*128+p, :] for n in 0..NT
    cosf = np.ascontiguousarray(
        cos[0, :, 0, :].reshape(NT, P, 32).transpose(1, 0, 2).reshape(P, NT * 32)
    ).astype(bf)
    sinf = np.ascontiguousarray(
        sin[0, :, 0, :].reshape(NT, P, 32).transpose(1, 0, 2).reshape(P, NT * 32)
    ).astype(bf)
    # trin[c, k] = -3e4 where key k > query c (strict upper): additive mask
    # accumulated into the diagonal score band via trin^T (identity moving).
    trin = np.where(
        np.arange(P)[None, :] > np.arange(P)[:, None], -30000.0, 0.0
    ).astype(bf)
    in_maps = []
    for c in range(8):
        b, g = c // N_KV_HEAD, c % N_KV_HEAD
        xTc = np.ascontiguousarray(x[b].T).astype(bf)  # [C, T]
        wq = Wq[g * 256 : (g + 1) * 256, :]           # [256, C]
        wk = Wk[g * D : (g + 1) * D, :]               # [64, C]
        wv = Wv[g * D : (g + 1) * D, :]
        wg_col = np.zeros((C, 1), np.float32)
        wg_col[:12, 0] = Wg[g]
        wrc = np.concatenate(
            [wq.T, wk.T, wv.T, wg_col, np.zeros((C, 1), np.float32)], axis=1
        ).astype(bf)                                  # [C, 386]
        ve3 = np.ascontiguousarray(
            (3.0 * ve[b, :, g * D : (g + 1) * D])
            .reshape(NT, P, D).transpose(1, 0, 2).reshape(P, NT * D)
        ).astype(bf)                                  # [P, NT*64]
        woTc = np.ascontiguousarray(
            Wo[:, g * 256 : (g + 1) * 256].T
        ).astype(bf)                                  # [256, C]
        in_maps.append(
            {
                "xT": xTc,
                "wr": np.ascontiguousarray(wrc),
                "cosd": cosf,
                "sind": sinf,
                "ve3": ve3,
                "woT": woTc,
                "trind": trin,
            }
        )
    return in_maps


_PROGRAM = None


def kernel(x, ve, cos, sin, Wq, Wk, Wv, Wo, Wg, _trace=False):
    from concourse.bass_utils import run_bass_kernel_spmd

    # coerce to host fp32 ndarrays up front (harness may pass jax arrays)
    x, ve, cos, sin, Wq, Wk, Wv, Wo, Wg = (
        np.asarray(a, dtype=np.float32)
        for a in (x, ve, cos, sin, Wq, Wk, Wv, Wo, Wg)
    )
    global _PROGRAM
    if _PROGRAM is None:
        _PROGRAM = build_program()
    nc = _PROGRAM
    in_maps = make_core_inputs(x, ve, cos, sin, Wq, Wk, Wv, Wo, Wg)
    res = run_bass_kernel_spmd(nc, in_maps, list(range(8)), trace=_trace)
    outs = [r["out"] for r in res.results]
    full = np.zeros((B, T, C), np.float32)
    for c in range(8):
        full[c // N_KV_HEAD] += np.asarray(outs[c], dtype=np.float32)
    if _trace:
        kernel.last_results = res
    return full



# revision 70
# speedup vs baseline: 1.0134x; 1.0134x over previous
"""Trainium2 Bass kernel for nn_Attention_5299989643989.

GQA attention forward (B=2, T=2048, C=1024, 16 q heads / 4 kv heads, D=64)
with value-embedding gating, rotary embedding, qk rms-norm, causal softmax.

Sharding: 8 cores = batch (2) x kv-head-group (4).  Each core computes its
4 q heads / 1 kv head end-to-end plus the Wo row-shard partial output; the
host sums the 4 partials per batch (the Wo all-reduce, done at unshard).

Per-core structure (bf16 data paths, fp32 PSUM accumulate; ~141us/core in
the TRN2 cost-model timeline at rel err 5.7e-3, vs 175us/2.6e-4 for the
all-fp32r ancestor). Emission order is tuned so the score/exp stream owns
the in-order engine-queue heads: the next group's projections and the
previous group's normalize/Wo are emitted INSIDE phase2 via a per-head
hook (deferral placement is performance-sensitive).
  phase1a: per 128-token chunk one jammed projection matmul
           [q(256)|k(64)|v(64)|gate(1)|pad] over bf16 x/W accumulated in
           PSUM and copied to a bf16 SBUF group tile; per 4-chunk group:
           rope over all 20 head instances via 4D strided bf16 views (2x
           DVE), rms rstd via bit-trick + ONE Newton step on DVE (bf16
           squares), per-chunk normalize multiplies so each chunk's
           transposes unblock early, one batched sigmoid-gate Tanh,
           ve-gating on GPSIMD.
  phase1b: paired 2-head bf16 PE transposes (1 cycle/row) into qT
           [128,2,T] and row-duplicated kT2 [128,T] (matmul requires equal
           stationary/moving partition bases).
  phase2:  per (head, 512-query block): scoresT tiles [128 keys, <=512
           live queries] = kT^T q; the causal mask is a -3e4 additive
           mask-matmul (trin^T x identity_bf16) accumulated into the
           diagonal PSUM band so exp underflows masked lanes to exactly 0
           (no Pool multiply, no extra engine hop); exp on ACT (the one
           saturated engine mid-run) with the folded 1.2*1.2/sqrt(64)
           scale, bf16 out; yT [65,512] += v_aug^T expT with a ones column
           producing denominators for free; 4-slot PSUM score pipeline
           with depth-4 AV stagger, 9 ex buffers.
  norm3:   denominator reciprocals on DVE, f32r rounding copy (Pool
           mid-run / ACT in the drain tail), PE outer-product broadcast,
           yT scaling, row-sharded bf16 Wo, per-chunk [128,1024] staging
           tile with PSUM readouts alternating DVE/ACT in the tail, one
           DMA per token chunk.
  Software pipeline at emission: group bi+1's projections and DVE chain
  are emitted before phase2(bi); normalize/Wo of bi-1 fill the PE while
  bi's transposes wait on the DVE chain; the last group's pair-0
  normalize is emitted mid-phase2 so only pair 1 + Wo remain in the tail.
Host side ships bf16 inputs (x^T, W-jam, cos/sin, 3*ve, Wo^T, mask) and
upcasts the bf16 per-core partials while summing the Wo row-shards.
"""

import numpy as np

import concourse.bacc as bacc
import concourse.bass as bass
import concourse.tile as tile
from concourse import mybir
from concourse.masks import make_identity

f32 = mybir.dt.float32
f32r = mybir.dt.float32r
bf16 = mybir.dt.bfloat16
AF = mybir.ActivationFunctionType

B, T, C = 2, 2048, 1024
N_HEAD, N_KV_HEAD, D = 16, 4, 64
HQ = N_HEAD // N_KV_HEAD  # q heads per core = 4
P = 128
NT = T // P       # 16 token chunks
KC = C // P       # 8 contraction chunks
IB = 512          # query block
NBI = T // IB     # 4 query blocks
GRP = IB // P     # 4 token chunks per query block
SC = 1.2 * 1.2 / 8.0  # folded qk scale: rms 1.2 factors * 1/sqrt(64)
H32 = D // 2


def build_program():
    nc = bacc.Bacc("TRN2", target_bir_lowering=False, debug=False, num_devices=8)

    xT = nc.dram_tensor("xT", [C, T], bf16, kind="ExternalInput")
    wr = nc.dram_tensor("wr", [C, 386], bf16, kind="ExternalInput")
    # cos/sin/ve3 pre-swizzled on the host into the SBUF-resident layout so
    # the DMA moves long contiguous rows (512B+ descriptors, full bandwidth)
    cosd = nc.dram_tensor("cosd", [P, NT * 32], bf16, kind="ExternalInput")
    sind = nc.dram_tensor("sind", [P, NT * 32], bf16, kind="ExternalInput")
    ve3 = nc.dram_tensor("ve3", [P, NT * D], bf16, kind="ExternalInput")
    woT = nc.dram_tensor("woT", [2 * P, C], bf16, kind="ExternalInput")
    trind = nc.dram_tensor("trind", [P, P], bf16, kind="ExternalInput")
    out = nc.dram_tensor("out", [T, C], bf16, kind="ExternalOutput")

    with tile.TileContext(nc) as tc:
        with (
            tc.tile_pool(name="consts", bufs=1) as consts,
            tc.tile_pool(name="resid", bufs=1) as resid,
            tc.tile_pool(name="xload", bufs=2) as xload,
            tc.tile_pool(name="rot", bufs=2) as rot,
            tc.tile_pool(name="small", bufs=4) as small,
            tc.tile_pool(name="exps", bufs=9) as exps,
            tc.tile_pool(name="ynp", bufs=5) as ynp,
            tc.tile_pool(name="outsb", bufs=4) as outsb,
            tc.tile_pool(name="psmm", bufs=2, space="PSUM") as psmm,
            tc.tile_pool(name="pssc", bufs=4, space="PSUM") as pssc,
            tc.tile_pool(name="psy", bufs=2, space="PSUM") as psy,
        ):
            # ---- resident loads ----
            # The cost-model DMA lane is serial (~0.003 ns/B), so order
            # strictly by need: weights + cos/sin first (small), then x block
            # 0 token-major in 4 pieces so each projection chunk can run as
            # its tokens arrive, then everything else.
            wr_sb = consts.tile([P, KC, 386], bf16)
            nc.sync.dma_start(
                wr_sb[:, 0, :],
                wr[0:P, :],
            )
            nc.sync.dma_start(
                wr_sb[:, 1:KC, :],
                wr[P:C, :].rearrange("(kc p) c -> p kc c", p=P),
            )
            xt0 = xload.tile([P, KC, IB], bf16, name="xt0", tag="xt")
            nc.sync.dma_start(
                xt0[:, 0 : KC // 2, :],
                xT[0 : C // 2, 0:IB].rearrange("(kc p) t -> p kc t", p=P),
            )
            nc.sync.dma_start(
                xt0[:, KC // 2 : KC, :],
                xT[C // 2 : C, 0:IB].rearrange("(kc p) t -> p kc t", p=P),
            )
            cos_sb = consts.tile([P, NT, 32], bf16)
            nc.sync.dma_start(cos_sb[:].rearrange("p n d -> p (n d)"), cosd[:])
            sin_sb = consts.tile([P, NT, 32], bf16)
            nc.sync.dma_start(sin_sb[:].rearrange("p n d -> p (n d)"), sind[:])
            xt1 = xload.tile([P, KC, IB], bf16, name="xt1", tag="xt")
            nc.sync.dma_start(
                xt1[:],
                xT[:, IB : 2 * IB].rearrange("(kc p) t -> p kc t", p=P),
            )
            ve3_sb = consts.tile([P, NT, D], bf16)
            nc.sync.dma_start(ve3_sb[:].rearrange("p n d -> p (n d)"), ve3[:])
            trin_sb = consts.tile([P, P], bf16)
            nc.sync.dma_start(trin_sb[:], trind[:])
            wo1_sb = consts.tile([P, C], bf16)
            nc.sync.dma_start(wo1_sb[:], woT[0:P, :])
            wo2_sb = consts.tile([P, C], bf16)
            nc.sync.dma_start(wo2_sb[:], woT[P : 2 * P, :])
            ident = consts.tile([P, P], f32)
            make_identity(nc, ident[:])
            identb = consts.tile([P, P], bf16)
            nc.vector.tensor_copy(identb[:], ident[:])
            # PE p-state warmup: keep the PE streaming through the initial
            # DMA wait so the first projection matmuls run at full clock
            # (cold PE costs 1.5-3.7x per matmul until 3us of busy ramp)
            warm = psy.tile([P, P], f32, tag="y")
            for _ in range(44):
                nc.tensor.matmul(warm[:], identb[:], identb[:])
            rsq_k = consts.tile([P, 1], mybir.dt.uint32)
            nc.vector.memset(rsq_k[:], 0x5F3759DF)
            zero_sb = consts.tile([P, 1], f32)
            nc.vector.memset(zero_sb[:], 0.0)

            # ---- residents written by the kernel ----
            qT = resid.tile([P, 2, T], bf16)   # [h0|h1] rows, [h2|h3] rows
            kT2 = resid.tile([P, T], bf16)     # kT duplicated in both row halves
            v_aug = resid.tile([P, NT, D + 1], bf16)  # v plus ones column
            nc.vector.memset(v_aug[:, :, D : D + 1], 1.0)
            yT1 = resid.tile([P, T], bf16)        # yT heads 0,1
            yT2 = resid.tile([P, T], bf16)        # yT heads 2,3

            def load_x(bi, q=None):
                xt = xload.tile([P, KC, IB], bf16, name=f"xt{bi}", tag="xt")
                (q or nc.scalar).dma_start(
                    xt[:],
                    xT[:, bi * IB : (bi + 1) * IB]
                    .rearrange("(kc p) t -> p kc t", p=P),
                )
                return xt

            def phase1(bi, xt, halves=1):
                qkr = rot.tile([P, GRP, 320], bf16, tag="qkr", bufs=1)  # roped q|k
                pjg = rot.tile([P, GRP, 386], bf16, tag="pjg")
                tgg = small.tile([P, GRP], f32, tag="tgg")
                for tl in range(GRP):
                    tc_ = bi * GRP + tl
                    pj = psmm.tile([P, 512], f32, tag="mm")
                    for kc in range(KC):
                        nc.tensor.matmul(
                            pj[:, 0:386],
                            xt[:, kc, tl * P : (tl + 1) * P],
                            wr_sb[:, kc, :],
                            start=(kc == 0),
                            stop=(kc == KC - 1),
                        )
                    # ACT for the startup groups (latency-critical chain);
                    # DVE for the mid-run groups where ACT is exp-saturated
                    if bi < 2:
                        nc.scalar.copy(pjg[:, tl, :], pj[:, 0:386])
                    else:
                        nc.vector.tensor_copy(pjg[:, tl, :], pj[:, 0:386])
                # one tanh over all 4 chunks' gate column (already staged in
                # the pjg copy) instead of 4 per-chunk PSUM reads
                nc.scalar.activation(
                    tgg[:], pjg[:, :, 384], AF.Tanh,
                    scale=0.5, bias=zero_sb[:],
                )

                # rope + rms + rstd + normalize over the group, optionally in
                # two chunk-pair halves (shorter DVE chain before the
                # transposes, at the cost of ~16 extra small DVE ops).
                tmp = rot.tile([P, GRP, 160], bf16, tag="tmp", bufs=1)
                sqg = rot.tile([P, GRP, 320], bf16, tag="sqg", bufs=1)
                msg = small.tile([P, GRP * 5], f32, tag="msg")
                rstdg = small.tile([P, GRP * 5], f32, tag="rstdg")
                nwt = small.tile([P, GRP * 5], f32, tag="nwt")
                qkn = rot.tile([P, GRP, 320], bf16, tag="qkn", bufs=2)
                gstep = GRP // halves
                for hf in range(halves):
                    g0, g1_ = gstep * hf, gstep * (hf + 1)
                    f0, f1 = 5 * gstep * hf, 5 * gstep * (hf + 1)
                    nf = f1 - f0
                    qv5 = pjg[:, g0:g1_, 0:320].rearrange(
                        "p g (h d) -> p g h d", d=D
                    )
                    ro5 = qkr[:, g0:g1_, :].rearrange("p g (h d) -> p g h d", d=D)
                    t5 = tmp[:, g0:g1_, :].rearrange("p g (h d) -> p g h d", d=H32)
                    cs = cos_sb[:, bi * GRP + g0 : bi * GRP + g1_, :]
                    sn = sin_sb[:, bi * GRP + g0 : bi * GRP + g1_, :]
                    cos5 = cs.unsqueeze(2).broadcast_to([P, g1_ - g0, 5, H32])
                    sin5 = sn.unsqueeze(2).broadcast_to([P, g1_ - g0, 5, H32])
                    q1 = qv5[:, :, :, 0:H32]
                    q2 = qv5[:, :, :, H32:D]
                    nc.vector.tensor_mul(ro5[:, :, :, 0:H32], q1, cos5)
                    nc.vector.tensor_mul(t5[:], q2, sin5)
                    nc.vector.tensor_add(
                        ro5[:, :, :, 0:H32], ro5[:, :, :, 0:H32], t5[:]
                    )
                    nc.vector.tensor_mul(ro5[:, :, :, H32:D], q2, cos5)
                    nc.vector.tensor_mul(t5[:], q1, sin5)
                    nc.vector.tensor_sub(
                        ro5[:, :, :, H32:D], ro5[:, :, :, H32:D], t5[:]
                    )

                    nc.vector.tensor_mul(
                        sqg[:, g0:g1_, :], qkr[:, g0:g1_, :], qkr[:, g0:g1_, :]
                    )
                    nc.vector.reduce_sum(
                        msg[:, f0:f1],
                        sqg[:, g0:g1_, :].rearrange("p g (h d) -> p (g h) d", d=D),
                        axis=mybir.AxisListType.X,
                    )
                    # m = mean + eps; rstd = m^-1/2 by bit-trick seed + two
                    # Newton iterations, entirely on DVE (no ACT Ln table).
                    nc.vector.tensor_scalar(
                        msg[:, f0:f1], msg[:, f0:f1], 1.0 / D, 1e-6,
                        op0=mybir.AluOpType.mult, op1=mybir.AluOpType.add,
                    )
                    rstdu = rstdg[:, f0:f1].bitcast(mybir.dt.uint32)
                    nc.vector.tensor_scalar(
                        rstdu, msg[:, f0:f1].bitcast(mybir.dt.uint32), 1, None,
                        op0=mybir.AluOpType.logical_shift_right,
                    )
                    nc.vector.tensor_sub(
                        rstdu,
                        rsq_k[:].broadcast_to([P, nf]).bitcast(mybir.dt.uint32),
                        rstdu,
                    )
                    # one Newton step suffices: 3.4% seed error -> ~0.2%,
                    # well inside the bf16 data path's noise floor
                    for _ in range(1):
                        nc.vector.tensor_mul(
                            nwt[:, f0:f1], msg[:, f0:f1], rstdg[:, f0:f1]
                        )
                        nc.vector.tensor_mul(
                            nwt[:, f0:f1], nwt[:, f0:f1], rstdg[:, f0:f1]
                        )
                        nc.vector.tensor_scalar(
                            nwt[:, f0:f1], nwt[:, f0:f1], -0.5, 1.5,
                            op0=mybir.AluOpType.mult, op1=mybir.AluOpType.add,
                        )
                        nc.vector.tensor_mul(
                            rstdg[:, f0:f1], rstdg[:, f0:f1], nwt[:, f0:f1]
                        )
                    # normalize per token chunk so each chunk's transposes
                    # unblock as soon as its multiply lands
                    for g in range(g0, g1_):
                        nc.vector.tensor_mul(
                            qkn[:, g, :].rearrange("p (h d) -> p h d", d=D),
                            qkr[:, g, :].rearrange("p (h d) -> p h d", d=D),
                            rstdg[:, 5 * g : 5 * (g + 1)]
                            .unsqueeze(2)
                            .broadcast_to([P, 5, D]),
                        )
                # gate r = sigmoid(z) = 0.5 + 0.5*tanh(z/2); ve3 is 3*ve.
                # Emitted after the rms chain so the DVE reaches the chain
                # sooner; elementwise v work runs on the idle GPSIMD.
                rgg = small.tile([P, GRP], f32, tag="rgg")
                nc.vector.tensor_scalar(
                    rgg[:], tgg[:], 0.5, 0.5,
                    op0=mybir.AluOpType.mult, op1=mybir.AluOpType.add,
                )
                vtg = small.tile([P, GRP, D], f32, tag="vtg", bufs=1)
                nc.gpsimd.tensor_mul(
                    vtg[:],
                    ve3_sb[:, bi * GRP : (bi + 1) * GRP, :],
                    rgg[:].unsqueeze(2).broadcast_to([P, GRP, D]),
                )
                nc.gpsimd.tensor_add(
                    v_aug[:, bi * GRP : (bi + 1) * GRP, 0:D],
                    pjg[:, :, 320:384],
                    vtg[:],
                )
                qkns[bi] = qkn

            def phase1b(bi):
                qkn = qkns.pop(bi)
                # transposes (two heads per [128,128] transpose)
                tpk = pssc.tile([D, 512], bf16, tag="sc")
                for tl in range(GRP):
                    tc_ = bi * GRP + tl
                    tp = pssc.tile([P, 256], bf16, tag="sc")
                    nc.tensor.transpose(
                        tp[:, 0:P], qkn[:, tl, 0:128], identb[:]
                    )
                    nc.tensor.transpose(
                        tp[:, P : 2 * P], qkn[:, tl, 128:256], identb[:]
                    )
                    nc.tensor.transpose(
                        tpk[:, tl * P : (tl + 1) * P], qkn[:, tl, 256:320], identb[:]
                    )
                    nc.vector.tensor_copy(
                        qT[:, :, tc_ * P : (tc_ + 1) * P],
                        tp[:].rearrange("p (g t) -> p g t", g=2),
                    )
                nc.vector.tensor_copy(kT2[0:D, bi * IB : (bi + 1) * IB], tpk[:])
                nc.vector.tensor_copy(kT2[D:P, bi * IB : (bi + 1) * IB], tpk[:])

            def phase2(bi, after_head=None):
                yns = []
                for h in range(HQ):
                    # y accumulated [query, qc, d | den]: one PSUM bank, four
                    # per-qc accumulation regions.  Only (jt=0, qc=0) starts;
                    # the bank-wide pending-zero makes the other qc's first
                    # write a plain store (skip_group_check for the regions).
                    yp = psy.tile([P, GRP, D + 1], f32, tag="y")
                    njt = GRP * (bi + 1)
                    rr = D * (h % 2)
                    qTh = qT[rr : rr + D, h // 2, :]

                    nfull = GRP * bi + 1  # tiles with lo == 0
                    pending = []

                    def score_mm(spc, jt):
                        dg = jt - GRP * bi
                        lo = max(dg, 0) * P
                        nc.tensor.matmul(
                            spc[:, lo:512],
                            kT2[rr : rr + D, jt * P : (jt + 1) * P],
                            qTh[:, bi * IB + lo : (bi + 1) * IB],
                            start=True,
                            stop=(dg < 0),
                        )
                        if dg >= 0:
                            # additive causal mask: -3e4 above the diagonal so
                            # exp underflows to exactly 0 (no Pool multiply)
                            nc.tensor.matmul(
                                spc[:, lo : lo + P],
                                trin_sb[:],
                                identb[:],
                                start=False,
                                stop=True,
                            )
                        return lo, dg

                    def emit_av(jt, exap, dg):
                        # stationary = exp scores [128 keys, 128 queries],
                        # moving = v_aug [128 keys, 65]: 65-col streams (the
                        # ldweights swap is free) instead of 512-col streams
                        for qc in range(GRP):
                            if dg > qc:
                                continue  # whole qc block above the diagonal
                            nc.tensor.matmul(
                                yp[:, qc, :],
                                exap[:, qc * P : (qc + 1) * P],
                                v_aug[:, jt, :],
                                start=(jt == 0 and qc == 0),
                                stop=(jt == GRP * bi + qc),
                                skip_group_check=True,
                            )

                    def flush(n):
                        while len(pending) > n:
                            emit_av(*pending.pop(0))

                    for jt in range(njt):
                        sp = pssc.tile([P, 512], f32, tag="sc", name="sp")
                        ex = exps.tile([P, 512], bf16, tag="ex", name="ex")
                        lo, dg = score_mm(sp, jt)
                        nc.scalar.activation(
                            ex[:, lo:512], sp[:, lo:512], AF.Exp,
                            scale=SC, bias=zero_sb[:],
                        )
                        pending.append((jt, ex, dg))
                        flush(6)
                    flush(0)
                    # the last head's hook (next group's phase1b) fires
                    # before its normalize tail so the qT/kT2 copies get
                    # ahead of the tail ops in the in-order DVE queue: the
                    # next block's first scores depend on them
                    if after_head is not None:
                        after_head(h)
                        fired_last = True
                    else:
                        fired_last = False
                    # normalize in [query, d] orientation: per-partition
                    # denominator scalars.  The PE transposes into the yT
                    # residents are deferred to the end of the bi block so the
                    # in-order PE queue never parks on a transpose whose yn
                    # input is still deep in the DVE queue (that would starve
                    # the score stream and the exp pipeline behind it).
                    rec = small.tile([P, GRP], f32, tag="rec")
                    nc.vector.reciprocal_approx_fast(rec[:], yp[:, :, D])
                    # hybrid normalize: one DVE bulk copy out of PSUM, then
                    # the per-qc scalar multiplies on the idle Pool engine
                    yn = ynp.tile([P, GRP, D], bf16, tag="yn")
                    if bi == NBI - 1 and h == HQ - 1:
                        # drain tail: direct DVE normalize, shortest chain
                        for qc in range(GRP):
                            nc.vector.tensor_scalar(
                                yn[:, qc, :], yp[:, qc, 0:D], rec[:, qc : qc + 1],
                                None, op0=mybir.AluOpType.mult,
                            )
                    else:
                        # hybrid: one DVE bulk copy out of PSUM, then the
                        # per-qc scalar multiplies on the idle Pool engine
                        yc = ynp.tile([P, GRP, D], bf16, tag="yc")
                        nc.vector.tensor_copy(yc[:], yp[:, :, 0:D])
                        for qc in range(GRP):
                            nc.gpsimd.tensor_scalar(
                                yn[:, qc, :], yc[:, qc, :], rec[:, qc : qc + 1],
                                None, op0=mybir.AluOpType.mult,
                            )
                    yns.append((h, yn))
                    if after_head is not None and not fired_last:
                        after_head(h)
                for h, yn in yns:
                    stg = psy.tile([D, IB], bf16, tag="y")
                    for qc in range(GRP):
                        nc.tensor.transpose(
                            stg[:, qc * P : (qc + 1) * P], yn[:, qc, :], identb[:]
                        )
                    ytp = yT1 if h < 2 else yT2
                    row = D * (h % 2)
                    nc.vector.tensor_copy(
                        ytp[row : row + D, bi * IB : (bi + 1) * IB], stg[:]
                    )

            def norm3w(bi, split_copies=False):
                # in the drain tail the score pool is free: 4 po slots keep
                # the Wo stream, readout copies and out-DMAs fully pipelined
                for tl in range(GRP):
                    tc_ = bi * GRP + tl
                    ob = outsb.tile([P, C], bf16, tag="ob")
                    for cb in range(2):
                        if split_copies:
                            po = pssc.tile([P, 512], f32, tag="sc")
                        else:
                            po = psmm.tile([P, 512], f32, tag="mm")
                        nc.tensor.matmul(
                            po[:],
                            yT1[:, tc_ * P : (tc_ + 1) * P],
                            wo1_sb[:, cb * 512 : (cb + 1) * 512],
                            start=True,
                            stop=False,
                        )
                        nc.tensor.matmul(
                            po[:],
                            yT2[:, tc_ * P : (tc_ + 1) * P],
                            wo2_sb[:, cb * 512 : (cb + 1) * 512],
                            start=False,
                            stop=True,
                        )
                        # in the drain tail ACT is idle: alternate the PSUM
                        # readout between DVE and ACT so po slots recycle 2x
                        # faster
                        if split_copies and cb == 1:
                            nc.scalar.copy(ob[:, cb * 512 : (cb + 1) * 512], po[:])
                        else:
                            nc.vector.tensor_copy(
                                ob[:, cb * 512 : (cb + 1) * 512], po[:]
                            )
                    nc.sync.dma_start(out[tc_ * P : (tc_ + 1) * P, :], ob[:])

            # group-level software pipeline: next group's projections are
            # emitted before the previous group's Wo so the PE has ready work
            # while the per-head normalize chains resolve.
            xts = {0: xt0, 1: xt1}
            qkns = {}
            phase1(0, xts[0], halves=2)
            phase1b(0)

            def hook0(h):
                if h == 0:
                    phase1(1, xts[1])
                    xts[2] = load_x(2)
                elif h == 3:
                    phase1b(1)

            phase2(0, after_head=hook0)
            for bi in range(1, NBI):
                # the next group's projections, transposes and bi-1's
                # Wo/writeout are deferred into phase2 via the per-head hook
                # so they do not sit ahead of the score/exp stream in the
                # in-order queues, and so the transposes finish well before
                # the group boundary
                def hook(h, bi=bi):
                    if h == 0:
                        if bi + 1 < NBI:
                            phase1(bi + 1, xts[bi + 1])
                            if bi + 2 < NBI:
                                xts[bi + 2] = load_x(bi + 2)
                    elif h == 1:
                        norm3w(bi - 1)
                    elif h == 3 and bi + 1 < NBI:
                        phase1b(bi + 1)

                phase2(bi, after_head=hook)
                if bi == NBI - 1:
                    norm3w(bi, split_copies=True)
    nc.compile()
    return nc


def make_core_inputs(x, ve, cos, sin, Wq, Wk, Wv, Wo, Wg):
    """Slice full inputs into the 8 per-core input maps (b-major, then group)."""
    import ml_dtypes

    bf = ml_dtypes.bfloat16
    # device layout [P, NT*32]: row p holds cos[n*128+p, :] for n in 0..NT
    cosf = np.ascontiguousarray(
        cos[0, :, 0, :].reshape(NT, P, 32).transpose(1, 0, 2).reshape(P, NT * 32)
    ).astype(bf)
    sinf = np.ascontiguousarray(
        sin[0, :, 0, :].reshape(NT, P, 32).transpose(1, 0, 2).reshape(P, NT * 32)
    ).astype(bf)
    # trin[c, k] = -3e4 where key k > query c (strict upper): additive mask
    # accumulated into the diagonal score band via trin^T (identity moving).
    trin = np.where(
        np.arange(P)[None, :] > np.arange(P)[:, None], -30000.0, 0.0
    ).astype(bf)
    in_maps = []
    for c in range(8):
        b, g = c // N_KV_HEAD, c % N_KV_HEAD
        xTc = np.ascontiguousarray(x[b].T).astype(bf)  # [C, T]
        wq = Wq[g * 256 : (g + 1) * 256, :]           # [256, C]
        wk = Wk[g * D : (g + 1) * D, :]               # [64, C]
        wv = Wv[g * D : (g + 1) * D, :]
        wg_col = np.zeros((C, 1), np.float32)
        wg_col[:12, 0] = Wg[g]
        wrc = np.concatenate(
            [wq.T, wk.T, wv.T, wg_col, np.zeros((C, 1), np.float32)], axis=1
        ).astype(bf)                                  # [C, 386]
        ve3 = np.ascontiguousarray(
            (3.0 * ve[b, :, g * D : (g + 1) * D])
            .reshape(NT, P, D).transpose(1, 0, 2).reshape(P, NT * D)
        ).astype(bf)                                  # [P, NT*64]
        woTc = np.ascontiguousarray(
            Wo[:, g * 256 : (g + 1) * 256].T
        ).astype(bf)                                  # [256, C]
        in_maps.append(
            {
                "xT": xTc,
                "wr": np.ascontiguousarray(wrc),
                "cosd": cosf,
                "sind": sinf,
                "ve3": ve3,
                "woT": woTc,
                "trind": trin,
            }
        )
    return in_maps


_PROGRAM = None


def kernel(x, ve, cos, sin, Wq, Wk, Wv, Wo, Wg, _trace=False):
    from concourse.bass_utils import run_bass_kernel_spmd

    # coerce to host fp32 ndarrays up front (harness may pass jax arrays)
    x, ve, cos, sin, Wq, Wk, Wv, Wo, Wg = (
        np.asarray(a, dtype=np.float32)
        for a in (x, ve, cos, sin, Wq, Wk, Wv, Wo, Wg)
    )
    global _PROGRAM
    if _PROGRAM is None:
        _PROGRAM = build_program()
    nc = _PROGRAM
    in_maps = make_core_inputs(x, ve, cos, sin, Wq, Wk, Wv, Wo, Wg)
    res = run_bass_kernel_spmd(nc, in_maps, list(range(8)), trace=_trace)
    outs = [r["out"] for r in res.results]
    full = np.zeros((B, T, C), np.float32)
    for c in range(8):
        full[c // N_KV_HEAD] += np.asarray(outs[c], dtype=np.float32)
    if _trace:
        kernel.last_results = res
    return full



# revision 71
# speedup vs baseline: 1.0136x; 1.0002x over previous
"""Trainium2 Bass kernel for nn_Attention_5299989643989.

GQA attention forward (B=2, T=2048, C=1024, 16 q heads / 4 kv heads, D=64)
with value-embedding gating, rotary embedding, qk rms-norm, causal softmax.

Sharding: 8 cores = batch (2) x kv-head-group (4).  Each core computes its
4 q heads / 1 kv head end-to-end plus the Wo row-shard partial output; the
host sums the 4 partials per batch (the Wo all-reduce, done at unshard).

Per-core structure (bf16 data paths, fp32 PSUM accumulate; ~141us/core in
the TRN2 cost-model timeline at rel err 5.7e-3, vs 175us/2.6e-4 for the
all-fp32r ancestor). Emission order is tuned so the score/exp stream owns
the in-order engine-queue heads: the next group's projections and the
previous group's normalize/Wo are emitted INSIDE phase2 via a per-head
hook (deferral placement is performance-sensitive).
  phase1a: per 128-token chunk one jammed projection matmul
           [q(256)|k(64)|v(64)|gate(1)|pad] over bf16 x/W accumulated in
           PSUM and copied to a bf16 SBUF group tile; per 4-chunk group:
           rope over all 20 head instances via 4D strided bf16 views (2x
           DVE), rms rstd via bit-trick + ONE Newton step on DVE (bf16
           squares), per-chunk normalize multiplies so each chunk's
           transposes unblock early, one batched sigmoid-gate Tanh,
           ve-gating on GPSIMD.
  phase1b: paired 2-head bf16 PE transposes (1 cycle/row) into qT
           [128,2,T] and row-duplicated kT2 [128,T] (matmul requires equal
           stationary/moving partition bases).
  phase2:  per (head, 512-query block): scoresT tiles [128 keys, <=512
           live queries] = kT^T q; the causal mask is a -3e4 additive
           mask-matmul (trin^T x identity_bf16) accumulated into the
           diagonal PSUM band so exp underflows masked lanes to exactly 0
           (no Pool multiply, no extra engine hop); exp on ACT (the one
           saturated engine mid-run) with the folded 1.2*1.2/sqrt(64)
           scale, bf16 out; yT [65,512] += v_aug^T expT with a ones column
           producing denominators for free; 4-slot PSUM score pipeline
           with depth-4 AV stagger, 9 ex buffers.
  norm3:   denominator reciprocals on DVE, f32r rounding copy (Pool
           mid-run / ACT in the drain tail), PE outer-product broadcast,
           yT scaling, row-sharded bf16 Wo, per-chunk [128,1024] staging
           tile with PSUM readouts alternating DVE/ACT in the tail, one
           DMA per token chunk.
  Software pipeline at emission: group bi+1's projections and DVE chain
  are emitted before phase2(bi); normalize/Wo of bi-1 fill the PE while
  bi's transposes wait on the DVE chain; the last group's pair-0
  normalize is emitted mid-phase2 so only pair 1 + Wo remain in the tail.
Host side ships bf16 inputs (x^T, W-jam, cos/sin, 3*ve, Wo^T, mask) and
upcasts the bf16 per-core partials while summing the Wo row-shards.
"""

import numpy as np

import concourse.bacc as bacc
import concourse.bass as bass
import concourse.tile as tile
from concourse import mybir
from concourse.masks import make_identity

f32 = mybir.dt.float32
f32r = mybir.dt.float32r
bf16 = mybir.dt.bfloat16
AF = mybir.ActivationFunctionType

B, T, C = 2, 2048, 1024
N_HEAD, N_KV_HEAD, D = 16, 4, 64
HQ = N_HEAD // N_KV_HEAD  # q heads per core = 4
P = 128
NT = T // P       # 16 token chunks
KC = C // P       # 8 contraction chunks
IB = 512          # query block
NBI = T // IB     # 4 query blocks
GRP = IB // P     # 4 token chunks per query block
SC = 1.2 * 1.2 / 8.0  # folded qk scale: rms 1.2 factors * 1/sqrt(64)
H32 = D // 2


def build_program():
    nc = bacc.Bacc("TRN2", target_bir_lowering=False, debug=False, num_devices=8)

    xT = nc.dram_tensor("xT", [C, T], bf16, kind="ExternalInput")
    wr = nc.dram_tensor("wr", [C, 386], bf16, kind="ExternalInput")
    # cos/sin/ve3 pre-swizzled on the host into the SBUF-resident layout so
    # the DMA moves long contiguous rows (512B+ descriptors, full bandwidth)
    cosd = nc.dram_tensor("cosd", [P, NT * 32], bf16, kind="ExternalInput")
    sind = nc.dram_tensor("sind", [P, NT * 32], bf16, kind="ExternalInput")
    ve3 = nc.dram_tensor("ve3", [P, NT * D], bf16, kind="ExternalInput")
    woT = nc.dram_tensor("woT", [2 * P, C], bf16, kind="ExternalInput")
    trind = nc.dram_tensor("trind", [P, P], bf16, kind="ExternalInput")
    out = nc.dram_tensor("out", [T, C], bf16, kind="ExternalOutput")

    with tile.TileContext(nc) as tc:
        with (
            tc.tile_pool(name="consts", bufs=1) as consts,
            tc.tile_pool(name="resid", bufs=1) as resid,
            tc.tile_pool(name="xload", bufs=2) as xload,
            tc.tile_pool(name="rot", bufs=2) as rot,
            tc.tile_pool(name="small", bufs=4) as small,
            tc.tile_pool(name="exps", bufs=9) as exps,
            tc.tile_pool(name="ynp", bufs=5) as ynp,
            tc.tile_pool(name="outsb", bufs=4) as outsb,
            tc.tile_pool(name="psmm", bufs=2, space="PSUM") as psmm,
            tc.tile_pool(name="pssc", bufs=4, space="PSUM") as pssc,
            tc.tile_pool(name="psy", bufs=2, space="PSUM") as psy,
        ):
            # ---- resident loads ----
            # The cost-model DMA lane is serial (~0.003 ns/B), so order
            # strictly by need: weights + cos/sin first (small), then x block
            # 0 token-major in 4 pieces so each projection chunk can run as
            # its tokens arrive, then everything else.
            wr_sb = consts.tile([P, KC, 386], bf16)
            nc.sync.dma_start(
                wr_sb[:, 0, :],
                wr[0:P, :],
            )
            nc.sync.dma_start(
                wr_sb[:, 1:KC, :],
                wr[P:C, :].rearrange("(kc p) c -> p kc c", p=P),
            )
            xt0 = xload.tile([P, KC, IB], bf16, name="xt0", tag="xt")
            nc.sync.dma_start(
                xt0[:, 0 : KC // 2, :],
                xT[0 : C // 2, 0:IB].rearrange("(kc p) t -> p kc t", p=P),
            )
            nc.sync.dma_start(
                xt0[:, KC // 2 : KC, :],
                xT[C // 2 : C, 0:IB].rearrange("(kc p) t -> p kc t", p=P),
            )
            cos_sb = consts.tile([P, NT, 32], bf16)
            nc.sync.dma_start(cos_sb[:].rearrange("p n d -> p (n d)"), cosd[:])
            sin_sb = consts.tile([P, NT, 32], bf16)
            nc.sync.dma_start(sin_sb[:].rearrange("p n d -> p (n d)"), sind[:])
            xt1 = xload.tile([P, KC, IB], bf16, name="xt1", tag="xt")
            nc.sync.dma_start(
                xt1[:],
                xT[:, IB : 2 * IB].rearrange("(kc p) t -> p kc t", p=P),
            )
            ve3_sb = consts.tile([P, NT, D], bf16)
            nc.sync.dma_start(ve3_sb[:].rearrange("p n d -> p (n d)"), ve3[:])
            trin_sb = consts.tile([P, P], bf16)
            nc.sync.dma_start(trin_sb[:], trind[:])
            wo1_sb = consts.tile([P, C], bf16)
            nc.sync.dma_start(wo1_sb[:], woT[0:P, :])
            wo2_sb = consts.tile([P, C], bf16)
            nc.sync.dma_start(wo2_sb[:], woT[P : 2 * P, :])
            ident = consts.tile([P, P], f32)
            make_identity(nc, ident[:])
            identb = consts.tile([P, P], bf16)
            nc.vector.tensor_copy(identb[:], ident[:])
            # PE p-state warmup: keep the PE streaming through the initial
            # DMA wait so the first projection matmuls run at full clock
            # (cold PE costs 1.5-3.7x per matmul until 3us of busy ramp)
            warm = psy.tile([P, P], f32, tag="y")
            for _ in range(44):
                nc.tensor.matmul(warm[:], identb[:], identb[:])
            rsq_k = consts.tile([P, 1], mybir.dt.uint32)
            nc.vector.memset(rsq_k[:], 0x5F3759DF)
            zero_sb = consts.tile([P, 1], f32)
            nc.vector.memset(zero_sb[:], 0.0)

            # ---- residents written by the kernel ----
            qT = resid.tile([P, 2, T], bf16)   # [h0|h1] rows, [h2|h3] rows
            kT2 = resid.tile([P, T], bf16)     # kT duplicated in both row halves
            v_aug = resid.tile([P, NT, D + 1], bf16)  # v plus ones column
            nc.vector.memset(v_aug[:, :, D : D + 1], 1.0)
            yT1 = resid.tile([P, T], bf16)        # yT heads 0,1
            yT2 = resid.tile([P, T], bf16)        # yT heads 2,3

            def load_x(bi, q=None):
                xt = xload.tile([P, KC, IB], bf16, name=f"xt{bi}", tag="xt")
                (q or nc.scalar).dma_start(
                    xt[:],
                    xT[:, bi * IB : (bi + 1) * IB]
                    .rearrange("(kc p) t -> p kc t", p=P),
                )
                return xt

            def phase1(bi, xt, halves=1):
                qkr = rot.tile([P, GRP, 320], bf16, tag="qkr", bufs=1)  # roped q|k
                pjg = rot.tile([P, GRP, 386], bf16, tag="pjg")
                tgg = small.tile([P, GRP], f32, tag="tgg")
                for tl in range(GRP):
                    tc_ = bi * GRP + tl
                    pj = psmm.tile([P, 512], f32, tag="mm")
                    for kc in range(KC):
                        nc.tensor.matmul(
                            pj[:, 0:386],
                            xt[:, kc, tl * P : (tl + 1) * P],
                            wr_sb[:, kc, :],
                            start=(kc == 0),
                            stop=(kc == KC - 1),
                        )
                    # ACT for the startup groups (latency-critical chain);
                    # DVE for the mid-run groups where ACT is exp-saturated
                    if bi < 2:
                        nc.scalar.copy(pjg[:, tl, :], pj[:, 0:386])
                    else:
                        nc.vector.tensor_copy(pjg[:, tl, :], pj[:, 0:386])
                # one tanh over all 4 chunks' gate column (already staged in
                # the pjg copy) instead of 4 per-chunk PSUM reads
                nc.scalar.activation(
                    tgg[:], pjg[:, :, 384], AF.Tanh,
                    scale=0.5, bias=zero_sb[:],
                )

                # rope + rms + rstd + normalize over the group, optionally in
                # two chunk-pair halves (shorter DVE chain before the
                # transposes, at the cost of ~16 extra small DVE ops).
                tmp = rot.tile([P, GRP, 160], bf16, tag="tmp", bufs=1)
                sqg = rot.tile([P, GRP, 320], bf16, tag="sqg", bufs=1)
                msg = small.tile([P, GRP * 5], f32, tag="msg")
                rstdg = small.tile([P, GRP * 5], f32, tag="rstdg")
                nwt = small.tile([P, GRP * 5], f32, tag="nwt")
                qkn = rot.tile([P, GRP, 320], bf16, tag="qkn", bufs=2)
                gstep = GRP // halves
                for hf in range(halves):
                    g0, g1_ = gstep * hf, gstep * (hf + 1)
                    f0, f1 = 5 * gstep * hf, 5 * gstep * (hf + 1)
                    nf = f1 - f0
                    qv5 = pjg[:, g0:g1_, 0:320].rearrange(
                        "p g (h d) -> p g h d", d=D
                    )
                    ro5 = qkr[:, g0:g1_, :].rearrange("p g (h d) -> p g h d", d=D)
                    t5 = tmp[:, g0:g1_, :].rearrange("p g (h d) -> p g h d", d=H32)
                    cs = cos_sb[:, bi * GRP + g0 : bi * GRP + g1_, :]
                    sn = sin_sb[:, bi * GRP + g0 : bi * GRP + g1_, :]
                    cos5 = cs.unsqueeze(2).broadcast_to([P, g1_ - g0, 5, H32])
                    sin5 = sn.unsqueeze(2).broadcast_to([P, g1_ - g0, 5, H32])
                    q1 = qv5[:, :, :, 0:H32]
                    q2 = qv5[:, :, :, H32:D]
                    nc.vector.tensor_mul(ro5[:, :, :, 0:H32], q1, cos5)
                    nc.vector.tensor_mul(t5[:], q2, sin5)
                    nc.vector.tensor_add(
                        ro5[:, :, :, 0:H32], ro5[:, :, :, 0:H32], t5[:]
                    )
                    nc.vector.tensor_mul(ro5[:, :, :, H32:D], q2, cos5)
                    nc.vector.tensor_mul(t5[:], q1, sin5)
                    nc.vector.tensor_sub(
                        ro5[:, :, :, H32:D], ro5[:, :, :, H32:D], t5[:]
                    )

                    nc.vector.tensor_mul(
                        sqg[:, g0:g1_, :], qkr[:, g0:g1_, :], qkr[:, g0:g1_, :]
                    )
                    nc.vector.reduce_sum(
                        msg[:, f0:f1],
                        sqg[:, g0:g1_, :].rearrange("p g (h d) -> p (g h) d", d=D),
                        axis=mybir.AxisListType.X,
                    )
                    # m = mean + eps; rstd = m^-1/2 by bit-trick seed + two
                    # Newton iterations, entirely on DVE (no ACT Ln table).
                    nc.vector.tensor_scalar(
                        msg[:, f0:f1], msg[:, f0:f1], 1.0 / D, 1e-6,
                        op0=mybir.AluOpType.mult, op1=mybir.AluOpType.add,
                    )
                    rstdu = rstdg[:, f0:f1].bitcast(mybir.dt.uint32)
                    nc.vector.tensor_scalar(
                        rstdu, msg[:, f0:f1].bitcast(mybir.dt.uint32), 1, None,
                        op0=mybir.AluOpType.logical_shift_right,
                    )
                    nc.vector.tensor_sub(
                        rstdu,
                        rsq_k[:].broadcast_to([P, nf]).bitcast(mybir.dt.uint32),
                        rstdu,
                    )
                    # one Newton step suffices: 3.4% seed error -> ~0.2%,
                    # well inside the bf16 data path's noise floor
                    for _ in range(1):
                        nc.vector.tensor_mul(
                            nwt[:, f0:f1], msg[:, f0:f1], rstdg[:, f0:f1]
                        )
                        nc.vector.tensor_mul(
                            nwt[:, f0:f1], nwt[:, f0:f1], rstdg[:, f0:f1]
                        )
                        nc.vector.tensor_scalar(
                            nwt[:, f0:f1], nwt[:, f0:f1], -0.5, 1.5,
                            op0=mybir.AluOpType.mult, op1=mybir.AluOpType.add,
                        )
                        nc.vector.tensor_mul(
                            rstdg[:, f0:f1], rstdg[:, f0:f1], nwt[:, f0:f1]
                        )
                    # normalize per token chunk so each chunk's transposes
                    # unblock as soon as its multiply lands
                    for g in range(g0, g1_):
                        nc.vector.tensor_mul(
                            qkn[:, g, :].rearrange("p (h d) -> p h d", d=D),
                            qkr[:, g, :].rearrange("p (h d) -> p h d", d=D),
                            rstdg[:, 5 * g : 5 * (g + 1)]
                            .unsqueeze(2)
                            .broadcast_to([P, 5, D]),
                        )
                # gate r = sigmoid(z) = 0.5 + 0.5*tanh(z/2); ve3 is 3*ve.
                # Emitted after the rms chain so the DVE reaches the chain
                # sooner; elementwise v work runs on the idle GPSIMD.
                rgg = small.tile([P, GRP], f32, tag="rgg")
                nc.vector.tensor_scalar(
                    rgg[:], tgg[:], 0.5, 0.5,
                    op0=mybir.AluOpType.mult, op1=mybir.AluOpType.add,
                )
                vtg = small.tile([P, GRP, D], f32, tag="vtg", bufs=1)
                nc.gpsimd.tensor_mul(
                    vtg[:],
                    ve3_sb[:, bi * GRP : (bi + 1) * GRP, :],
                    rgg[:].unsqueeze(2).broadcast_to([P, GRP, D]),
                )
                nc.gpsimd.tensor_add(
                    v_aug[:, bi * GRP : (bi + 1) * GRP, 0:D],
                    pjg[:, :, 320:384],
                    vtg[:],
                )
                qkns[bi] = qkn

            def phase1b(bi):
                qkn = qkns.pop(bi)
                # transposes (two heads per [128,128] transpose)
                tpk = pssc.tile([D, 512], bf16, tag="sc")
                for tl in range(GRP):
                    tc_ = bi * GRP + tl
                    tp = pssc.tile([P, 256], bf16, tag="sc")
                    nc.tensor.transpose(
                        tp[:, 0:P], qkn[:, tl, 0:128], identb[:]
                    )
                    nc.tensor.transpose(
                        tp[:, P : 2 * P], qkn[:, tl, 128:256], identb[:]
                    )
                    nc.tensor.transpose(
                        tpk[:, tl * P : (tl + 1) * P], qkn[:, tl, 256:320], identb[:]
                    )
                    nc.vector.tensor_copy(
                        qT[:, :, tc_ * P : (tc_ + 1) * P],
                        tp[:].rearrange("p (g t) -> p g t", g=2),
                    )
                nc.vector.tensor_copy(kT2[0:D, bi * IB : (bi + 1) * IB], tpk[:])
                nc.vector.tensor_copy(kT2[D:P, bi * IB : (bi + 1) * IB], tpk[:])

            def phase2(bi, after_head=None):
                yns = []
                for h in range(HQ):
                    # y accumulated [query, qc, d | den]: one PSUM bank, four
                    # per-qc accumulation regions.  Only (jt=0, qc=0) starts;
                    # the bank-wide pending-zero makes the other qc's first
                    # write a plain store (skip_group_check for the regions).
                    yp = psy.tile([P, GRP, D + 1], f32, tag="y")
                    njt = GRP * (bi + 1)
                    rr = D * (h % 2)
                    qTh = qT[rr : rr + D, h // 2, :]

                    nfull = GRP * bi + 1  # tiles with lo == 0
                    pending = []

                    def score_mm(spc, jt):
                        dg = jt - GRP * bi
                        lo = max(dg, 0) * P
                        nc.tensor.matmul(
                            spc[:, lo:512],
                            kT2[rr : rr + D, jt * P : (jt + 1) * P],
                            qTh[:, bi * IB + lo : (bi + 1) * IB],
                            start=True,
                            stop=(dg < 0),
                        )
                        if dg >= 0:
                            # additive causal mask: -3e4 above the diagonal so
                            # exp underflows to exactly 0 (no Pool multiply)
                            nc.tensor.matmul(
                                spc[:, lo : lo + P],
                                trin_sb[:],
                                identb[:],
                                start=False,
                                stop=True,
                            )
                        return lo, dg

                    def emit_av(jt, exap, dg):
                        # stationary = exp scores [128 keys, 128 queries],
                        # moving = v_aug [128 keys, 65]: 65-col streams (the
                        # ldweights swap is free) instead of 512-col streams
                        for qc in range(GRP):
                            if dg > qc:
                                continue  # whole qc block above the diagonal
                            nc.tensor.matmul(
                                yp[:, qc, :],
                                exap[:, qc * P : (qc + 1) * P],
                                v_aug[:, jt, :],
                                start=(jt == 0 and qc == 0),
                                stop=(jt == GRP * bi + qc),
                                skip_group_check=True,
                            )

                    def flush(n):
                        while len(pending) > n:
                            emit_av(*pending.pop(0))

                    for jt in range(njt):
                        sp = pssc.tile([P, 512], f32, tag="sc", name="sp")
                        ex = exps.tile([P, 512], bf16, tag="ex", name="ex")
                        lo, dg = score_mm(sp, jt)
                        nc.scalar.activation(
                            ex[:, lo:512], sp[:, lo:512], AF.Exp,
                            scale=SC, bias=zero_sb[:],
                        )
                        pending.append((jt, ex, dg))
                        flush(4)
                    flush(0)
                    # the last head's hook (next group's phase1b) fires
                    # before its normalize tail so the qT/kT2 copies get
                    # ahead of the tail ops in the in-order DVE queue: the
                    # next block's first scores depend on them
                    if after_head is not None:
                        after_head(h)
                        fired_last = True
                    else:
                        fired_last = False
                    # normalize in [query, d] orientation: per-partition
                    # denominator scalars.  The PE transposes into the yT
                    # residents are deferred to the end of the bi block so the
                    # in-order PE queue never parks on a transpose whose yn
                    # input is still deep in the DVE queue (that would starve
                    # the score stream and the exp pipeline behind it).
                    rec = small.tile([P, GRP], f32, tag="rec")
                    nc.vector.reciprocal_approx_fast(rec[:], yp[:, :, D])
                    # hybrid normalize: one DVE bulk copy out of PSUM, then
                    # the per-qc scalar multiplies on the idle Pool engine
                    yn = ynp.tile([P, GRP, D], bf16, tag="yn")
                    if bi == NBI - 1 and h == HQ - 1:
                        # drain tail: direct DVE normalize, shortest chain
                        for qc in range(GRP):
                            nc.vector.tensor_scalar(
                                yn[:, qc, :], yp[:, qc, 0:D], rec[:, qc : qc + 1],
                                None, op0=mybir.AluOpType.mult,
                            )
                    else:
                        # hybrid: one DVE bulk copy out of PSUM, then the
                        # per-qc scalar multiplies on the idle Pool engine
                        yc = ynp.tile([P, GRP, D], bf16, tag="yc")
                        nc.vector.tensor_copy(yc[:], yp[:, :, 0:D])
                        for qc in range(GRP):
                            nc.gpsimd.tensor_scalar(
                                yn[:, qc, :], yc[:, qc, :], rec[:, qc : qc + 1],
                                None, op0=mybir.AluOpType.mult,
                            )
                    yns.append((h, yn))
                    if after_head is not None and not fired_last:
                        after_head(h)
                for h, yn in yns:
                    stg = psy.tile([D, IB], bf16, tag="y")
                    for qc in range(GRP):
                        nc.tensor.transpose(
                            stg[:, qc * P : (qc + 1) * P], yn[:, qc, :], identb[:]
                        )
                    ytp = yT1 if h < 2 else yT2
                    row = D * (h % 2)
                    nc.vector.tensor_copy(
                        ytp[row : row + D, bi * IB : (bi + 1) * IB], stg[:]
                    )

            def norm3w(bi, split_copies=False):
                # in the drain tail the score pool is free: 4 po slots keep
                # the Wo stream, readout copies and out-DMAs fully pipelined
                for tl in range(GRP):
                    tc_ = bi * GRP + tl
                    ob = outsb.tile([P, C], bf16, tag="ob")
                    for cb in range(2):
                        if split_copies:
                            po = pssc.tile([P, 512], f32, tag="sc")
                        else:
                            po = psmm.tile([P, 512], f32, tag="mm")
                        nc.tensor.matmul(
                            po[:],
                            yT1[:, tc_ * P : (tc_ + 1) * P],
                            wo1_sb[:, cb * 512 : (cb + 1) * 512],
                            start=True,
                            stop=False,
                        )
                        nc.tensor.matmul(
                            po[:],
                            yT2[:, tc_ * P : (tc_ + 1) * P],
                            wo2_sb[:, cb * 512 : (cb + 1) * 512],
                            start=False,
                            stop=True,
                        )
                        # in the drain tail ACT is idle: alternate the PSUM
                        # readout between DVE and ACT so po slots recycle 2x
                        # faster
                        if split_copies and cb == 1:
                            nc.scalar.copy(ob[:, cb * 512 : (cb + 1) * 512], po[:])
                        else:
                            nc.vector.tensor_copy(
                                ob[:, cb * 512 : (cb + 1) * 512], po[:]
                            )
                    nc.sync.dma_start(out[tc_ * P : (tc_ + 1) * P, :], ob[:])

            # group-level software pipeline: next group's projections are
            # emitted before the previous group's Wo so the PE has ready work
            # while the per-head normalize chains resolve.
            xts = {0: xt0, 1: xt1}
            qkns = {}
            phase1(0, xts[0], halves=2)
            phase1b(0)

            def hook0(h):
                if h == 0:
                    phase1(1, xts[1])
                    xts[2] = load_x(2)
                elif h == 3:
                    phase1b(1)

            phase2(0, after_head=hook0)
            for bi in range(1, NBI):
                # the next group's projections, transposes and bi-1's
                # Wo/writeout are deferred into phase2 via the per-head hook
                # so they do not sit ahead of the score/exp stream in the
                # in-order queues, and so the transposes finish well before
                # the group boundary
                def hook(h, bi=bi):
                    if h == 0:
                        if bi + 1 < NBI:
                            phase1(bi + 1, xts[bi + 1])
                            if bi + 2 < NBI:
                                xts[bi + 2] = load_x(bi + 2)
                    elif h == 1:
                        norm3w(bi - 1)
                    elif h == 3 and bi + 1 < NBI:
                        phase1b(bi + 1)

                phase2(bi, after_head=hook)
                if bi == NBI - 1:
                    norm3w(bi, split_copies=True)
    nc.compile()
    return nc


def make_core_inputs(x, ve, cos, sin, Wq, Wk, Wv, Wo, Wg):
    """Slice full inputs into the 8 per-core input maps (b-major, then group)."""
    import ml_dtypes

    bf = ml_dtypes.bfloat16
    # device layout [P, NT*32]: row p holds cos[n*128+p, :] for n in 0..NT
    cosf = np.ascontiguousarray(
        cos[0, :, 0, :].reshape(NT, P, 32).transpose(1, 0, 2).reshape(P, NT * 32)
    ).astype(bf)
    sinf = np.ascontiguousarray(
        sin[0, :, 0, :].reshape(NT, P, 32).transpose(1, 0, 2).reshape(P, NT * 32)
    ).astype(bf)
    # trin[c, k] = -3e4 where key k > query c (strict upper): additive mask
    # accumulated into the diagonal score band via trin^T (identity moving).
    trin = np.where(
        np.arange(P)[None, :] > np.arange(P)[:, None], -30000.0, 0.0
    ).astype(bf)
    in_maps = []
    for c in range(8):
        b, g = c // N_KV_HEAD, c % N_KV_HEAD
        xTc = np.ascontiguousarray(x[b].T).astype(bf)  # [C, T]
        wq = Wq[g * 256 : (g + 1) * 256, :]           # [256, C]
        wk = Wk[g * D : (g + 1) * D, :]               # [64, C]
        wv = Wv[g * D : (g + 1) * D, :]
        wg_col = np.zeros((C, 1), np.float32)
        wg_col[:12, 0] = Wg[g]
        wrc = np.concatenate(
            [wq.T, wk.T, wv.T, wg_col, np.zeros((C, 1), np.float32)], axis=1
        ).astype(bf)                                  # [C, 386]
        ve3 = np.ascontiguousarray(
            (3.0 * ve[b, :, g * D : (g + 1) * D])
            .reshape(NT, P, D).transpose(1, 0, 2).reshape(P, NT * D)
        ).astype(bf)                                  # [P, NT*64]
        woTc = np.ascontiguousarray(
            Wo[:, g * 256 : (g + 1) * 256].T
        ).astype(bf)                                  # [256, C]
        in_maps.append(
            {
                "xT": xTc,
                "wr": np.ascontiguousarray(wrc),
                "cosd": cosf,
                "sind": sinf,
                "ve3": ve3,
                "woT": woTc,
                "trind": trin,
            }
        )
    return in_maps


_PROGRAM = None


def kernel(x, ve, cos, sin, Wq, Wk, Wv, Wo, Wg, _trace=False):
    from concourse.bass_utils import run_bass_kernel_spmd

    # coerce to host fp32 ndarrays up front (harness may pass jax arrays)
    x, ve, cos, sin, Wq, Wk, Wv, Wo, Wg = (
        np.asarray(a, dtype=np.float32)
        for a in (x, ve, cos, sin, Wq, Wk, Wv, Wo, Wg)
    )
    global _PROGRAM
    if _PROGRAM is None:
        _PROGRAM = build_program()
    nc = _PROGRAM
    in_maps = make_core_inputs(x, ve, cos, sin, Wq, Wk, Wv, Wo, Wg)
    res = run_bass_kernel_spmd(nc, in_maps, list(range(8)), trace=_trace)
    outs = [r["out"] for r in res.results]
    full = np.zeros((B, T, C), np.float32)
    for c in range(8):
        full[c // N_KV_HEAD] += np.asarray(outs[c], dtype=np.float32)
    if _trace:
        kernel.last_results = res
    return full



# revision 74
# speedup vs baseline: 1.0137x; 1.0001x over previous
"""Trainium2 Bass kernel for nn_Attention_5299989643989.

GQA attention forward (B=2, T=2048, C=1024, 16 q heads / 4 kv heads, D=64)
with value-embedding gating, rotary embedding, qk rms-norm, causal softmax.

Sharding: 8 cores = batch (2) x kv-head-group (4).  Each core computes its
4 q heads / 1 kv head end-to-end plus the Wo row-shard partial output; the
host sums the 4 partials per batch (the Wo all-reduce, done at unshard).

Per-core structure (bf16 data paths, fp32 PSUM accumulate; ~141us/core in
the TRN2 cost-model timeline at rel err 5.7e-3, vs 175us/2.6e-4 for the
all-fp32r ancestor). Emission order is tuned so the score/exp stream owns
the in-order engine-queue heads: the next group's projections and the
previous group's normalize/Wo are emitted INSIDE phase2 via a per-head
hook (deferral placement is performance-sensitive).
  phase1a: per 128-token chunk one jammed projection matmul
           [q(256)|k(64)|v(64)|gate(1)|pad] over bf16 x/W accumulated in
           PSUM and copied to a bf16 SBUF group tile; per 4-chunk group:
           rope over all 20 head instances via 4D strided bf16 views (2x
           DVE), rms rstd via bit-trick + ONE Newton step on DVE (bf16
           squares), per-chunk normalize multiplies so each chunk's
           transposes unblock early, one batched sigmoid-gate Tanh,
           ve-gating on GPSIMD.
  phase1b: paired 2-head bf16 PE transposes (1 cycle/row) into qT
           [128,2,T] and row-duplicated kT2 [128,T] (matmul requires equal
           stationary/moving partition bases).
  phase2:  per (head, 512-query block): scoresT tiles [128 keys, <=512
           live queries] = kT^T q; the causal mask is a -3e4 additive
           mask-matmul (trin^T x identity_bf16) accumulated into the
           diagonal PSUM band so exp underflows masked lanes to exactly 0
           (no Pool multiply, no extra engine hop); exp on ACT (the one
           saturated engine mid-run) with the folded 1.2*1.2/sqrt(64)
           scale, bf16 out; yT [65,512] += v_aug^T expT with a ones column
           producing denominators for free; 4-slot PSUM score pipeline
           with depth-4 AV stagger, 9 ex buffers.
  norm3:   denominator reciprocals on DVE, f32r rounding copy (Pool
           mid-run / ACT in the drain tail), PE outer-product broadcast,
           yT scaling, row-sharded bf16 Wo, per-chunk [128,1024] staging
           tile with PSUM readouts alternating DVE/ACT in the tail, one
           DMA per token chunk.
  Software pipeline at emission: group bi+1's projections and DVE chain
  are emitted before phase2(bi); normalize/Wo of bi-1 fill the PE while
  bi's transposes wait on the DVE chain; the last group's pair-0
  normalize is emitted mid-phase2 so only pair 1 + Wo remain in the tail.
Host side ships bf16 inputs (x^T, W-jam, cos/sin, 3*ve, Wo^T, mask) and
upcasts the bf16 per-core partials while summing the Wo row-shards.
"""

import numpy as np

import concourse.bacc as bacc
import concourse.bass as bass
import concourse.tile as tile
from concourse import mybir
from concourse.masks import make_identity

f32 = mybir.dt.float32
f32r = mybir.dt.float32r
bf16 = mybir.dt.bfloat16
AF = mybir.ActivationFunctionType

B, T, C = 2, 2048, 1024
N_HEAD, N_KV_HEAD, D = 16, 4, 64
HQ = N_HEAD // N_KV_HEAD  # q heads per core = 4
P = 128
NT = T // P       # 16 token chunks
KC = C // P       # 8 contraction chunks
IB = 512          # query block
NBI = T // IB     # 4 query blocks
GRP = IB // P     # 4 token chunks per query block
SC = 1.2 * 1.2 / 8.0  # folded qk scale: rms 1.2 factors * 1/sqrt(64)
H32 = D // 2


def build_program():
    nc = bacc.Bacc("TRN2", target_bir_lowering=False, debug=False, num_devices=8)

    xT = nc.dram_tensor("xT", [C, T], bf16, kind="ExternalInput")
    wr = nc.dram_tensor("wr", [C, 386], bf16, kind="ExternalInput")
    # cos/sin/ve3 pre-swizzled on the host into the SBUF-resident layout so
    # the DMA moves long contiguous rows (512B+ descriptors, full bandwidth)
    cosd = nc.dram_tensor("cosd", [P, NT * 32], bf16, kind="ExternalInput")
    sind = nc.dram_tensor("sind", [P, NT * 32], bf16, kind="ExternalInput")
    ve3 = nc.dram_tensor("ve3", [P, NT * D], bf16, kind="ExternalInput")
    woT = nc.dram_tensor("woT", [2 * P, C], bf16, kind="ExternalInput")
    trind = nc.dram_tensor("trind", [P, P], bf16, kind="ExternalInput")
    out = nc.dram_tensor("out", [T, C], bf16, kind="ExternalOutput")

    with tile.TileContext(nc) as tc:
        with (
            tc.tile_pool(name="consts", bufs=1) as consts,
            tc.tile_pool(name="resid", bufs=1) as resid,
            tc.tile_pool(name="xload", bufs=2) as xload,
            tc.tile_pool(name="rot", bufs=2) as rot,
            tc.tile_pool(name="small", bufs=4) as small,
            tc.tile_pool(name="exps", bufs=9) as exps,
            tc.tile_pool(name="ynp", bufs=5) as ynp,
            tc.tile_pool(name="outsb", bufs=4) as outsb,
            tc.tile_pool(name="psmm", bufs=2, space="PSUM") as psmm,
            tc.tile_pool(name="pssc", bufs=4, space="PSUM") as pssc,
            tc.tile_pool(name="psy", bufs=2, space="PSUM") as psy,
        ):
            # ---- resident loads ----
            # The cost-model DMA lane is serial (~0.003 ns/B), so order
            # strictly by need: weights + cos/sin first (small), then x block
            # 0 token-major in 4 pieces so each projection chunk can run as
            # its tokens arrive, then everything else.
            wr_sb = consts.tile([P, KC, 386], bf16)
            nc.sync.dma_start(
                wr_sb[:, 0, :],
                wr[0:P, :],
            )
            nc.sync.dma_start(
                wr_sb[:, 1:KC, :],
                wr[P:C, :].rearrange("(kc p) c -> p kc c", p=P),
            )
            xt0 = xload.tile([P, KC, IB], bf16, name="xt0", tag="xt")
            nc.sync.dma_start(
                xt0[:, 0 : KC // 2, :],
                xT[0 : C // 2, 0:IB].rearrange("(kc p) t -> p kc t", p=P),
            )
            nc.sync.dma_start(
                xt0[:, KC // 2 : KC, :],
                xT[C // 2 : C, 0:IB].rearrange("(kc p) t -> p kc t", p=P),
            )
            cos_sb = consts.tile([P, NT, 32], bf16)
            nc.sync.dma_start(cos_sb[:].rearrange("p n d -> p (n d)"), cosd[:])
            sin_sb = consts.tile([P, NT, 32], bf16)
            nc.sync.dma_start(sin_sb[:].rearrange("p n d -> p (n d)"), sind[:])
            xt1 = xload.tile([P, KC, IB], bf16, name="xt1", tag="xt")
            nc.sync.dma_start(
                xt1[:],
                xT[:, IB : 2 * IB].rearrange("(kc p) t -> p kc t", p=P),
            )
            ve3_sb = consts.tile([P, NT, D], bf16)
            nc.sync.dma_start(ve3_sb[:].rearrange("p n d -> p (n d)"), ve3[:])
            trin_sb = consts.tile([P, P], bf16)
            nc.sync.dma_start(trin_sb[:], trind[:])
            wo1_sb = consts.tile([P, C], bf16)
            nc.sync.dma_start(wo1_sb[:], woT[0:P, :])
            wo2_sb = consts.tile([P, C], bf16)
            nc.sync.dma_start(wo2_sb[:], woT[P : 2 * P, :])
            ident = consts.tile([P, P], f32)
            make_identity(nc, ident[:])
            identb = consts.tile([P, P], bf16)
            nc.vector.tensor_copy(identb[:], ident[:])
            # PE p-state warmup: keep the PE streaming through the initial
            # DMA wait so the first projection matmuls run at full clock
            # (cold PE costs 1.5-3.7x per matmul until 3us of busy ramp)
            warm = psy.tile([P, P], f32, tag="y")
            for _ in range(44):
                nc.tensor.matmul(warm[:], identb[:], identb[:])
            rsq_k = consts.tile([P, 1], mybir.dt.uint32)
            nc.vector.memset(rsq_k[:], 0x5F3759DF)
            zero_sb = consts.tile([P, 1], f32)
            nc.vector.memset(zero_sb[:], 0.0)

            # ---- residents written by the kernel ----
            qT = resid.tile([P, 2, T], bf16)   # [h0|h1] rows, [h2|h3] rows
            kT2 = resid.tile([P, T], bf16)     # kT duplicated in both row halves
            v_aug = resid.tile([P, NT, D + 1], bf16)  # v plus ones column
            nc.vector.memset(v_aug[:, :, D : D + 1], 1.0)
            yT1 = resid.tile([P, T], bf16)        # yT heads 0,1
            yT2 = resid.tile([P, T], bf16)        # yT heads 2,3

            def load_x(bi, q=None):
                xt = xload.tile([P, KC, IB], bf16, name=f"xt{bi}", tag="xt")
                (q or nc.scalar).dma_start(
                    xt[:],
                    xT[:, bi * IB : (bi + 1) * IB]
                    .rearrange("(kc p) t -> p kc t", p=P),
                )
                return xt

            pjgs = {}

            def phase1_proj(bi, xt, tls):
                if bi not in pjgs:
                    pjgs[bi] = rot.tile([P, GRP, 386], bf16, tag="pjg", name=f"pjg{bi}")
                pjg = pjgs[bi]
                for tl in tls:
                    tc_ = bi * GRP + tl
                    pj = psmm.tile([P, 512], f32, tag="mm")
                    for kc in range(KC):
                        nc.tensor.matmul(
                            pj[:, 0:386],
                            xt[:, kc, tl * P : (tl + 1) * P],
                            wr_sb[:, kc, :],
                            start=(kc == 0),
                            stop=(kc == KC - 1),
                        )
                    # ACT for the startup groups (latency-critical chain);
                    # DVE for the mid-run groups where ACT is exp-saturated
                    if bi < 2:
                        nc.scalar.copy(pjg[:, tl, :], pj[:, 0:386])
                    else:
                        nc.vector.tensor_copy(pjg[:, tl, :], pj[:, 0:386])

            def phase1(bi, xt, halves=1, proj=True):
                if proj:
                    phase1_proj(bi, xt, range(GRP))
                qkr = rot.tile([P, GRP, 320], bf16, tag="qkr", bufs=1)  # roped q|k
                pjg = pjgs.pop(bi)
                tgg = small.tile([P, GRP], f32, tag="tgg")
                # one tanh over all 4 chunks' gate column (already staged in
                # the pjg copy) instead of 4 per-chunk PSUM reads
                nc.scalar.activation(
                    tgg[:], pjg[:, :, 384], AF.Tanh,
                    scale=0.5, bias=zero_sb[:],
                )

                # rope + rms + rstd + normalize over the group, optionally in
                # two chunk-pair halves (shorter DVE chain before the
                # transposes, at the cost of ~16 extra small DVE ops).
                tmp = rot.tile([P, GRP, 160], bf16, tag="tmp", bufs=1)
                sqg = rot.tile([P, GRP, 320], bf16, tag="sqg", bufs=1)
                msg = small.tile([P, GRP * 5], f32, tag="msg")
                rstdg = small.tile([P, GRP * 5], f32, tag="rstdg")
                nwt = small.tile([P, GRP * 5], f32, tag="nwt")
                qkn = rot.tile([P, GRP, 320], bf16, tag="qkn", bufs=2)
                gstep = GRP // halves
                for hf in range(halves):
                    g0, g1_ = gstep * hf, gstep * (hf + 1)
                    f0, f1 = 5 * gstep * hf, 5 * gstep * (hf + 1)
                    nf = f1 - f0
                    qv5 = pjg[:, g0:g1_, 0:320].rearrange(
                        "p g (h d) -> p g h d", d=D
                    )
                    ro5 = qkr[:, g0:g1_, :].rearrange("p g (h d) -> p g h d", d=D)
                    t5 = tmp[:, g0:g1_, :].rearrange("p g (h d) -> p g h d", d=H32)
                    cs = cos_sb[:, bi * GRP + g0 : bi * GRP + g1_, :]
                    sn = sin_sb[:, bi * GRP + g0 : bi * GRP + g1_, :]
                    cos5 = cs.unsqueeze(2).broadcast_to([P, g1_ - g0, 5, H32])
                    sin5 = sn.unsqueeze(2).broadcast_to([P, g1_ - g0, 5, H32])
                    q1 = qv5[:, :, :, 0:H32]
                    q2 = qv5[:, :, :, H32:D]
                    nc.vector.tensor_mul(ro5[:, :, :, 0:H32], q1, cos5)
                    nc.vector.tensor_mul(t5[:], q2, sin5)
                    nc.vector.tensor_add(
                        ro5[:, :, :, 0:H32], ro5[:, :, :, 0:H32], t5[:]
                    )
                    nc.vector.tensor_mul(ro5[:, :, :, H32:D], q2, cos5)
                    nc.vector.tensor_mul(t5[:], q1, sin5)
                    nc.vector.tensor_sub(
                        ro5[:, :, :, H32:D], ro5[:, :, :, H32:D], t5[:]
                    )

                    nc.vector.tensor_mul(
                        sqg[:, g0:g1_, :], qkr[:, g0:g1_, :], qkr[:, g0:g1_, :]
                    )
                    nc.vector.reduce_sum(
                        msg[:, f0:f1],
                        sqg[:, g0:g1_, :].rearrange("p g (h d) -> p (g h) d", d=D),
                        axis=mybir.AxisListType.X,
                    )
                    # m = mean + eps; rstd = m^-1/2 by bit-trick seed + two
                    # Newton iterations, entirely on DVE (no ACT Ln table).
                    nc.vector.tensor_scalar(
                        msg[:, f0:f1], msg[:, f0:f1], 1.0 / D, 1e-6,
                        op0=mybir.AluOpType.mult, op1=mybir.AluOpType.add,
                    )
                    rstdu = rstdg[:, f0:f1].bitcast(mybir.dt.uint32)
                    nc.vector.tensor_scalar(
                        rstdu, msg[:, f0:f1].bitcast(mybir.dt.uint32), 1, None,
                        op0=mybir.AluOpType.logical_shift_right,
                    )
                    nc.vector.tensor_sub(
                        rstdu,
                        rsq_k[:].broadcast_to([P, nf]).bitcast(mybir.dt.uint32),
                        rstdu,
                    )
                    # one Newton step suffices: 3.4% seed error -> ~0.2%,
                    # well inside the bf16 data path's noise floor
                    for _ in range(1):
                        nc.vector.tensor_mul(
                            nwt[:, f0:f1], msg[:, f0:f1], rstdg[:, f0:f1]
                        )
                        nc.vector.tensor_mul(
                            nwt[:, f0:f1], nwt[:, f0:f1], rstdg[:, f0:f1]
                        )
                        nc.vector.tensor_scalar(
                            nwt[:, f0:f1], nwt[:, f0:f1], -0.5, 1.5,
                            op0=mybir.AluOpType.mult, op1=mybir.AluOpType.add,
                        )
                        nc.vector.tensor_mul(
                            rstdg[:, f0:f1], rstdg[:, f0:f1], nwt[:, f0:f1]
                        )
                    # normalize per token chunk so each chunk's transposes
                    # unblock as soon as its multiply lands
                    for g in range(g0, g1_):
                        nc.vector.tensor_mul(
                            qkn[:, g, :].rearrange("p (h d) -> p h d", d=D),
                            qkr[:, g, :].rearrange("p (h d) -> p h d", d=D),
                            rstdg[:, 5 * g : 5 * (g + 1)]
                            .unsqueeze(2)
                            .broadcast_to([P, 5, D]),
                        )
                # gate r = sigmoid(z) = 0.5 + 0.5*tanh(z/2); ve3 is 3*ve.
                # Emitted after the rms chain so the DVE reaches the chain
                # sooner; elementwise v work runs on the idle GPSIMD.
                rgg = small.tile([P, GRP], f32, tag="rgg")
                nc.vector.tensor_scalar(
                    rgg[:], tgg[:], 0.5, 0.5,
                    op0=mybir.AluOpType.mult, op1=mybir.AluOpType.add,
                )
                vtg = small.tile([P, GRP, D], f32, tag="vtg", bufs=1)
                nc.gpsimd.tensor_mul(
                    vtg[:],
                    ve3_sb[:, bi * GRP : (bi + 1) * GRP, :],
                    rgg[:].unsqueeze(2).broadcast_to([P, GRP, D]),
                )
                nc.gpsimd.tensor_add(
                    v_aug[:, bi * GRP : (bi + 1) * GRP, 0:D],
                    pjg[:, :, 320:384],
                    vtg[:],
                )
                qkns[bi] = qkn

            def phase1b(bi):
                qkn = qkns.pop(bi)
                # transposes (two heads per [128,128] transpose)
                tpk = pssc.tile([D, 512], bf16, tag="sc")
                for tl in range(GRP):
                    tc_ = bi * GRP + tl
                    tp = pssc.tile([P, 256], bf16, tag="sc")
                    nc.tensor.transpose(
                        tp[:, 0:P], qkn[:, tl, 0:128], identb[:]
                    )
                    nc.tensor.transpose(
                        tp[:, P : 2 * P], qkn[:, tl, 128:256], identb[:]
                    )
                    nc.tensor.transpose(
                        tpk[:, tl * P : (tl + 1) * P], qkn[:, tl, 256:320], identb[:]
                    )
                    nc.vector.tensor_copy(
                        qT[:, :, tc_ * P : (tc_ + 1) * P],
                        tp[:].rearrange("p (g t) -> p g t", g=2),
                    )
                nc.vector.tensor_copy(kT2[0:D, bi * IB : (bi + 1) * IB], tpk[:])
                nc.vector.tensor_copy(kT2[D:P, bi * IB : (bi + 1) * IB], tpk[:])

            def phase2(bi, after_head=None):
                yns = []
                for h in range(HQ):
                    # y accumulated [query, qc, d | den]: one PSUM bank, four
                    # per-qc accumulation regions.  Only (jt=0, qc=0) starts;
                    # the bank-wide pending-zero makes the other qc's first
                    # write a plain store (skip_group_check for the regions).
                    yp = psy.tile([P, GRP, D + 1], f32, tag="y")
                    njt = GRP * (bi + 1)
                    rr = D * (h % 2)
                    qTh = qT[rr : rr + D, h // 2, :]

                    nfull = GRP * bi + 1  # tiles with lo == 0
                    pending = []

                    def score_mm(spc, jt):
                        dg = jt - GRP * bi
                        lo = max(dg, 0) * P
                        nc.tensor.matmul(
                            spc[:, lo:512],
                            kT2[rr : rr + D, jt * P : (jt + 1) * P],
                            qTh[:, bi * IB + lo : (bi + 1) * IB],
                            start=True,
                            stop=(dg < 0),
                        )
                        if dg >= 0:
                            # additive causal mask: -3e4 above the diagonal so
                            # exp underflows to exactly 0 (no Pool multiply)
                            nc.tensor.matmul(
                                spc[:, lo : lo + P],
                                trin_sb[:],
                                identb[:],
                                start=False,
                                stop=True,
                            )
                        return lo, dg

                    def emit_av(jt, exap, dg):
                        # stationary = exp scores [128 keys, 128 queries],
                        # moving = v_aug [128 keys, 65]: 65-col streams (the
                        # ldweights swap is free) instead of 512-col streams
                        for qc in range(GRP):
                            if dg > qc:
                                continue  # whole qc block above the diagonal
                            nc.tensor.matmul(
                                yp[:, qc, :],
                                exap[:, qc * P : (qc + 1) * P],
                                v_aug[:, jt, :],
                                start=(jt == 0 and qc == 0),
                                stop=(jt == GRP * bi + qc),
                                skip_group_check=True,
                            )

                    def flush(n):
                        while len(pending) > n:
                            emit_av(*pending.pop(0))

                    for jt in range(njt):
                        sp = pssc.tile([P, 512], f32, tag="sc", name="sp")
                        ex = exps.tile([P, 512], bf16, tag="ex", name="ex")
                        lo, dg = score_mm(sp, jt)
                        nc.scalar.activation(
                            ex[:, lo:512], sp[:, lo:512], AF.Exp,
                            scale=SC, bias=zero_sb[:],
                        )
                        pending.append((jt, ex, dg))
                        flush(4)
                    flush(0)
                    # the last head's hook (next group's phase1b) fires
                    # before its normalize tail so the qT/kT2 copies get
                    # ahead of the tail ops in the in-order DVE queue: the
                    # next block's first scores depend on them
                    if after_head is not None:
                        after_head(h)
                        fired_last = True
                    else:
                        fired_last = False
                    # normalize in [query, d] orientation: per-partition
                    # denominator scalars.  The PE transposes into the yT
                    # residents are deferred to the end of the bi block so the
                    # in-order PE queue never parks on a transpose whose yn
                    # input is still deep in the DVE queue (that would starve
                    # the score stream and the exp pipeline behind it).
                    rec = small.tile([P, GRP], f32, tag="rec")
                    nc.vector.reciprocal_approx_fast(rec[:], yp[:, :, D])
                    # hybrid normalize: one DVE bulk copy out of PSUM, then
                    # the per-qc scalar multiplies on the idle Pool engine
                    yn = ynp.tile([P, GRP, D], bf16, tag="yn")
                    if bi == NBI - 1 and h == HQ - 1:
                        # drain tail: direct DVE normalize, shortest chain
                        for qc in range(GRP):
                            nc.vector.tensor_scalar(
                                yn[:, qc, :], yp[:, qc, 0:D], rec[:, qc : qc + 1],
                                None, op0=mybir.AluOpType.mult,
                            )
                    else:
                        # hybrid: one DVE bulk copy out of PSUM, then the
                        # per-qc scalar multiplies on the idle Pool engine
                        yc = ynp.tile([P, GRP, D], bf16, tag="yc")
                        nc.vector.tensor_copy(yc[:], yp[:, :, 0:D])
                        for qc in range(GRP):
                            nc.gpsimd.tensor_scalar(
                                yn[:, qc, :], yc[:, qc, :], rec[:, qc : qc + 1],
                                None, op0=mybir.AluOpType.mult,
                            )
                    yns.append((h, yn))
                    if after_head is not None and not fired_last:
                        after_head(h)
                for h, yn in yns:
                    stg = psy.tile([D, IB], bf16, tag="y")
                    for qc in range(GRP):
                        nc.tensor.transpose(
                            stg[:, qc * P : (qc + 1) * P], yn[:, qc, :], identb[:]
                        )
                    ytp = yT1 if h < 2 else yT2
                    row = D * (h % 2)
                    nc.vector.tensor_copy(
                        ytp[row : row + D, bi * IB : (bi + 1) * IB], stg[:]
                    )

            def norm3w(bi, split_copies=False):
                # in the drain tail the score pool is free: 4 po slots keep
                # the Wo stream, readout copies and out-DMAs fully pipelined
                for tl in range(GRP):
                    tc_ = bi * GRP + tl
                    ob = outsb.tile([P, C], bf16, tag="ob")
                    for cb in range(2):
                        if split_copies:
                            po = pssc.tile([P, 512], f32, tag="sc")
                        else:
                            po = psmm.tile([P, 512], f32, tag="mm")
                        nc.tensor.matmul(
                            po[:],
                            yT1[:, tc_ * P : (tc_ + 1) * P],
                            wo1_sb[:, cb * 512 : (cb + 1) * 512],
                            start=True,
                            stop=False,
                        )
                        nc.tensor.matmul(
                            po[:],
                            yT2[:, tc_ * P : (tc_ + 1) * P],
                            wo2_sb[:, cb * 512 : (cb + 1) * 512],
                            start=False,
                            stop=True,
                        )
                        # in the drain tail ACT is idle: alternate the PSUM
                        # readout between DVE and ACT so po slots recycle 2x
                        # faster
                        if split_copies and cb == 1:
                            nc.scalar.copy(ob[:, cb * 512 : (cb + 1) * 512], po[:])
                        else:
                            nc.vector.tensor_copy(
                                ob[:, cb * 512 : (cb + 1) * 512], po[:]
                            )
                    nc.sync.dma_start(out[tc_ * P : (tc_ + 1) * P, :], ob[:])

            # group-level software pipeline: next group's projections are
            # emitted before the previous group's Wo so the PE has ready work
            # while the per-head normalize chains resolve.
            xts = {0: xt0, 1: xt1}
            qkns = {}
            phase1(0, xts[0], halves=2)
            phase1b(0)

            def hook0(h):
                # group 1's projections split across the thin bi-0 heads so
                # the 5us proj block doesn't starve the exp stream
                if h == 0:
                    phase1_proj(1, xts[1], [0, 1])
                    xts[2] = load_x(2)
                elif h == 1:
                    phase1_proj(1, xts[1], [2, 3])
                    phase1(1, xts[1], proj=False)
                elif h == 3:
                    phase1b(1)

            phase2(0, after_head=hook0)
            for bi in range(1, NBI):
                # the next group's projections, transposes and bi-1's
                # Wo/writeout are deferred into phase2 via the per-head hook
                # so they do not sit ahead of the score/exp stream in the
                # in-order queues, and so the transposes finish well before
                # the group boundary
                def hook(h, bi=bi):
                    if h == 0:
                        if bi + 1 < NBI:
                            phase1(bi + 1, xts[bi + 1])
                            if bi + 2 < NBI:
                                xts[bi + 2] = load_x(bi + 2)
                    elif h == 1:
                        norm3w(bi - 1)
                    elif h == 3 and bi + 1 < NBI:
                        phase1b(bi + 1)

                phase2(bi, after_head=hook)
                if bi == NBI - 1:
                    norm3w(bi, split_copies=True)
    nc.compile()
    return nc


def make_core_inputs(x, ve, cos, sin, Wq, Wk, Wv, Wo, Wg):
    """Slice full inputs into the 8 per-core input maps (b-major, then group)."""
    import ml_dtypes

    bf = ml_dtypes.bfloat16
    # device layout [P, NT*32]: row p holds cos[n*128+p, :] for n in 0..NT
    cosf = np.ascontiguousarray(
        cos[0, :, 0, :].reshape(NT, P, 32).transpose(1, 0, 2).reshape(P, NT * 32)
    ).astype(bf)
    sinf = np.ascontiguousarray(
        sin[0, :, 0, :].reshape(NT, P, 32).transpose(1, 0, 2).reshape(P, NT * 32)
    ).astype(bf)
    # trin[c, k] = -3e4 where key k > query c (strict upper): additive mask
    # accumulated into the diagonal score band via trin^T (identity moving).
    trin = np.where(
        np.arange(P)[None, :] > np.arange(P)[:, None], -30000.0, 0.0
    ).astype(bf)
    in_maps = []
    for c in range(8):
        b, g = c // N_KV_HEAD, c % N_KV_HEAD
        xTc = np.ascontiguousarray(x[b].T).astype(bf)  # [C, T]
        wq = Wq[g * 256 : (g + 1) * 256, :]           # [256, C]
        wk = Wk[g * D : (g + 1) * D, :]               # [64, C]
        wv = Wv[g * D : (g + 1) * D, :]
        wg_col = np.zeros((C, 1), np.float32)
        wg_col[:12, 0] = Wg[g]
        wrc = np.concatenate(
            [wq.T, wk.T, wv.T, wg_col, np.zeros((C, 1), np.float32)], axis=1
        ).astype(bf)                                  # [C, 386]
        ve3 = np.ascontiguousarray(
            (3.0 * ve[b, :, g * D : (g + 1) * D])
            .reshape(NT, P, D).transpose(1, 0, 2).reshape(P, NT * D)
        ).astype(bf)                                  # [P, NT*64]
        woTc = np.ascontiguousarray(
            Wo[:, g * 256 : (g + 1) * 256].T
        ).astype(bf)                                  # [256, C]
        in_maps.append(
            {
                "xT": xTc,
                "wr": np.ascontiguousarray(wrc),
                "cosd": cosf,
                "sind": sinf,
                "ve3": ve3,
                "woT": woTc,
                "trind": trin,
            }
        )
    return in_maps


_PROGRAM = None


def kernel(x, ve, cos, sin, Wq, Wk, Wv, Wo, Wg, _trace=False):
    from concourse.bass_utils import run_bass_kernel_spmd

    # coerce to host fp32 ndarrays up front (harness may pass jax arrays)
    x, ve, cos, sin, Wq, Wk, Wv, Wo, Wg = (
        np.asarray(a, dtype=np.float32)
        for a in (x, ve, cos, sin, Wq, Wk, Wv, Wo, Wg)
    )
    global _PROGRAM
    if _PROGRAM is None:
        _PROGRAM = build_program()
    nc = _PROGRAM
    in_maps = make_core_inputs(x, ve, cos, sin, Wq, Wk, Wv, Wo, Wg)
    res = run_bass_kernel_spmd(nc, in_maps, list(range(8)), trace=_trace)
    outs = [r["out"] for r in res.results]
    full = np.zeros((B, T, C), np.float32)
    for c in range(8):
        full[c // N_KV_HEAD] += np.asarray(outs[c], dtype=np.float32)
    if _trace:
        kernel.last_results = res
    return full



# revision 84
# speedup vs baseline: 1.0287x; 1.0148x over previous
"""Trainium2 Bass kernel for nn_Attention_5299989643989.

GQA attention forward (B=2, T=2048, C=1024, 16 q heads / 4 kv heads, D=64)
with value-embedding gating, rotary embedding, qk rms-norm, causal softmax.

Sharding: 8 cores = batch (2) x kv-head-group (4).  Each core computes its
4 q heads / 1 kv head end-to-end plus the Wo row-shard partial output; the
host sums the 4 partials per batch (the Wo all-reduce, done at unshard).

Per-core structure (bf16 data paths, fp32 PSUM accumulate; ~141us/core in
the TRN2 cost-model timeline at rel err 5.7e-3, vs 175us/2.6e-4 for the
all-fp32r ancestor). Emission order is tuned so the score/exp stream owns
the in-order engine-queue heads: the next group's projections and the
previous group's normalize/Wo are emitted INSIDE phase2 via a per-head
hook (deferral placement is performance-sensitive).
  phase1a: per 128-token chunk one jammed projection matmul
           [q(256)|k(64)|v(64)|gate(1)|pad] over bf16 x/W accumulated in
           PSUM and copied to a bf16 SBUF group tile; per 4-chunk group:
           rope over all 20 head instances via 4D strided bf16 views (2x
           DVE), rms rstd via bit-trick + ONE Newton step on DVE (bf16
           squares), per-chunk normalize multiplies so each chunk's
           transposes unblock early, one batched sigmoid-gate Tanh,
           ve-gating on GPSIMD.
  phase1b: paired 2-head bf16 PE transposes (1 cycle/row) into qT
           [128,2,T] and row-duplicated kT2 [128,T] (matmul requires equal
           stationary/moving partition bases).
  phase2:  per (head, 512-query block): scoresT tiles [128 keys, <=512
           live queries] = kT^T q; the causal mask is a -3e4 additive
           mask-matmul (trin^T x identity_bf16) accumulated into the
           diagonal PSUM band so exp underflows masked lanes to exactly 0
           (no Pool multiply, no extra engine hop); exp on ACT (the one
           saturated engine mid-run) with the folded 1.2*1.2/sqrt(64)
           scale, bf16 out; yT [65,512] += v_aug^T expT with a ones column
           producing denominators for free; 4-slot PSUM score pipeline
           with depth-4 AV stagger, 9 ex buffers.
  norm3:   denominator reciprocals on DVE, f32r rounding copy (Pool
           mid-run / ACT in the drain tail), PE outer-product broadcast,
           yT scaling, row-sharded bf16 Wo, per-chunk [128,1024] staging
           tile with PSUM readouts alternating DVE/ACT in the tail, one
           DMA per token chunk.
  Software pipeline at emission: group bi+1's projections and DVE chain
  are emitted before phase2(bi); normalize/Wo of bi-1 fill the PE while
  bi's transposes wait on the DVE chain; the last group's pair-0
  normalize is emitted mid-phase2 so only pair 1 + Wo remain in the tail.
Host side ships bf16 inputs (x^T, W-jam, cos/sin, 3*ve, Wo^T, mask) and
upcasts the bf16 per-core partials while summing the Wo row-shards.
"""

import numpy as np

import concourse.bacc as bacc
import concourse.bass as bass
import concourse.tile as tile
from concourse import mybir
from concourse.masks import make_identity

f32 = mybir.dt.float32
f32r = mybir.dt.float32r
bf16 = mybir.dt.bfloat16
AF = mybir.ActivationFunctionType

B, T, C = 2, 2048, 1024
N_HEAD, N_KV_HEAD, D = 16, 4, 64
HQ = N_HEAD // N_KV_HEAD  # q heads per core = 4
P = 128
NT = T // P       # 16 token chunks
KC = C // P       # 8 contraction chunks
IB = 512          # query block
NBI = T // IB     # 4 query blocks
GRP = IB // P     # 4 token chunks per query block
SC = 1.2 * 1.2 / 8.0  # folded qk scale: rms 1.2 factors * 1/sqrt(64)
H32 = D // 2


def build_program():
    nc = bacc.Bacc("TRN2", target_bir_lowering=False, debug=False, num_devices=8)

    xT = nc.dram_tensor("xT", [C, T], bf16, kind="ExternalInput")
    wr = nc.dram_tensor("wr", [C, 386], bf16, kind="ExternalInput")
    # cos/sin/ve3 pre-swizzled on the host into the SBUF-resident layout so
    # the DMA moves long contiguous rows (512B+ descriptors, full bandwidth)
    cosd = nc.dram_tensor("cosd", [P, NT * 32], bf16, kind="ExternalInput")
    sind = nc.dram_tensor("sind", [P, NT * 32], bf16, kind="ExternalInput")
    ve3 = nc.dram_tensor("ve3", [P, NT * D], bf16, kind="ExternalInput")
    woT = nc.dram_tensor("woT", [2 * P, C], bf16, kind="ExternalInput")
    trind = nc.dram_tensor("trind", [P, P], bf16, kind="ExternalInput")
    out = nc.dram_tensor("out", [T, C], bf16, kind="ExternalOutput")

    with tile.TileContext(nc) as tc:
        with (
            tc.tile_pool(name="consts", bufs=1) as consts,
            tc.tile_pool(name="resid", bufs=1) as resid,
            tc.tile_pool(name="xload", bufs=2) as xload,
            tc.tile_pool(name="rot", bufs=2) as rot,
            tc.tile_pool(name="small", bufs=6) as small,
            tc.tile_pool(name="exps", bufs=9) as exps,
            tc.tile_pool(name="ynp", bufs=5) as ynp,
            tc.tile_pool(name="outsb", bufs=4) as outsb,
            tc.tile_pool(name="psmm", bufs=2, space="PSUM") as psmm,
            tc.tile_pool(name="pssc", bufs=4, space="PSUM") as pssc,
            tc.tile_pool(name="psy", bufs=2, space="PSUM") as psy,
        ):
            # ---- resident loads ----
            # The cost-model DMA lane is serial (~0.003 ns/B), so order
            # strictly by need: weights + cos/sin first (small), then x block
            # 0 token-major in 4 pieces so each projection chunk can run as
            # its tokens arrive, then everything else.
            wr_sb = consts.tile([P, KC, 386], bf16)
            nc.sync.dma_start(
                wr_sb[:, 0, :],
                wr[0:P, :],
            )
            nc.sync.dma_start(
                wr_sb[:, 1:KC, :],
                wr[P:C, :].rearrange("(kc p) c -> p kc c", p=P),
            )
            xt0 = xload.tile([P, KC, IB], bf16, name="xt0", tag="xt")
            nc.sync.dma_start(
                xt0[:, 0 : KC // 2, :],
                xT[0 : C // 2, 0:IB].rearrange("(kc p) t -> p kc t", p=P),
            )
            nc.sync.dma_start(
                xt0[:, KC // 2 : KC, :],
                xT[C // 2 : C, 0:IB].rearrange("(kc p) t -> p kc t", p=P),
            )
            cos_sb = consts.tile([P, NT, 32], bf16)
            nc.sync.dma_start(cos_sb[:].rearrange("p n d -> p (n d)"), cosd[:])
            sin_sb = consts.tile([P, NT, 32], bf16)
            nc.sync.dma_start(sin_sb[:].rearrange("p n d -> p (n d)"), sind[:])
            xt1 = xload.tile([P, KC, IB], bf16, name="xt1", tag="xt")
            nc.sync.dma_start(
                xt1[:],
                xT[:, IB : 2 * IB].rearrange("(kc p) t -> p kc t", p=P),
            )
            ve3_sb = consts.tile([P, NT, D], bf16)
            nc.sync.dma_start(ve3_sb[:].rearrange("p n d -> p (n d)"), ve3[:])
            trin_sb = consts.tile([P, P], bf16)
            nc.sync.dma_start(trin_sb[:], trind[:])
            wo1_sb = consts.tile([P, C], bf16)
            nc.sync.dma_start(wo1_sb[:], woT[0:P, :])
            wo2_sb = consts.tile([P, C], bf16)
            nc.sync.dma_start(wo2_sb[:], woT[P : 2 * P, :])
            ident = consts.tile([P, P], f32)
            make_identity(nc, ident[:])
            identb = consts.tile([P, P], bf16)
            nc.vector.tensor_copy(identb[:], ident[:])
            # PE p-state warmup: keep the PE streaming through the initial
            # DMA wait so the first projection matmuls run at full clock
            # (cold PE costs 1.5-3.7x per matmul until 3us of busy ramp)
            warm = psy.tile([P, P], f32, tag="y")
            for _ in range(44):
                nc.tensor.matmul(warm[:], identb[:], identb[:])
            rsq_k = consts.tile([P, 1], mybir.dt.uint32)
            nc.vector.memset(rsq_k[:], 0x5F3759DF)
            zero_sb = consts.tile([P, 1], f32)
            nc.vector.memset(zero_sb[:], 0.0)

            # ---- residents written by the kernel ----
            qT = resid.tile([P, 2, T], bf16)   # [h0|h1] rows, [h2|h3] rows
            kT2 = resid.tile([P, T], bf16)     # kT duplicated in both row halves
            v_aug = resid.tile([P, NT, D + 1], bf16)  # v plus ones column
            nc.vector.memset(v_aug[:, :, D : D + 1], 1.0)
            yT1 = resid.tile([P, T], bf16)        # yT heads 0,1
            yT2 = resid.tile([P, T], bf16)        # yT heads 2,3

            def load_x(bi, q=None):
                xt = xload.tile([P, KC, IB], bf16, name=f"xt{bi}", tag="xt")
                (q or nc.gpsimd).dma_start(
                    xt[:],
                    xT[:, bi * IB : (bi + 1) * IB]
                    .rearrange("(kc p) t -> p kc t", p=P),
                )
                return xt

            pjgs = {}

            def phase1_proj(bi, xt, tls):
                if bi not in pjgs:
                    pjgs[bi] = rot.tile([P, GRP, 386], bf16, tag="pjg", name=f"pjg{bi}")
                pjg = pjgs[bi]
                for tl in tls:
                    tc_ = bi * GRP + tl
                    pj = psmm.tile([P, 512], f32, tag="mm")
                    for kc in range(KC):
                        nc.tensor.matmul(
                            pj[:, 0:386],
                            xt[:, kc, tl * P : (tl + 1) * P],
                            wr_sb[:, kc, :],
                            start=(kc == 0),
                            stop=(kc == KC - 1),
                        )
                    # ACT for the startup groups (latency-critical chain);
                    # DVE for the mid-run groups where ACT is exp-saturated
                    if bi < 2:
                        nc.scalar.copy(pjg[:, tl, :], pj[:, 0:386])
                    else:
                        nc.vector.tensor_copy(pjg[:, tl, :], pj[:, 0:386])

            def phase1(bi, xt, halves=1, proj=True):
                if proj:
                    phase1_proj(bi, xt, range(GRP))
                qkr = rot.tile([P, GRP, 320], bf16, tag="qkr", bufs=1)  # roped q|k
                pjg = pjgs.pop(bi)
                tgg = small.tile([P, GRP], f32, tag="tgg")
                # one tanh over all 4 chunks' gate column (already staged in
                # the pjg copy) instead of 4 per-chunk PSUM reads
                nc.scalar.activation(
                    tgg[:], pjg[:, :, 384], AF.Tanh,
                    scale=0.5, bias=zero_sb[:],
                )

                # rope + rms + rstd + normalize over the group, optionally in
                # two chunk-pair halves (shorter DVE chain before the
                # transposes, at the cost of ~16 extra small DVE ops).
                tmp = rot.tile([P, GRP, 160], bf16, tag="tmp", bufs=1)
                sqg = rot.tile([P, GRP, 320], bf16, tag="sqg", bufs=1)
                msg = small.tile([P, GRP * 5], f32, tag="msg")
                rstdg = small.tile([P, GRP * 5], f32, tag="rstdg")
                nwt = small.tile([P, GRP * 5], f32, tag="nwt")
                qkn = rot.tile([P, GRP, 320], bf16, tag="qkn", bufs=2)
                gstep = GRP // halves
                for hf in range(halves):
                    g0, g1_ = gstep * hf, gstep * (hf + 1)
                    f0, f1 = 5 * gstep * hf, 5 * gstep * (hf + 1)
                    nf = f1 - f0
                    qv5 = pjg[:, g0:g1_, 0:320].rearrange(
                        "p g (h d) -> p g h d", d=D
                    )
                    ro5 = qkr[:, g0:g1_, :].rearrange("p g (h d) -> p g h d", d=D)
                    t5 = tmp[:, g0:g1_, :].rearrange("p g (h d) -> p g h d", d=H32)
                    cs = cos_sb[:, bi * GRP + g0 : bi * GRP + g1_, :]
                    sn = sin_sb[:, bi * GRP + g0 : bi * GRP + g1_, :]
                    cos5 = cs.unsqueeze(2).broadcast_to([P, g1_ - g0, 5, H32])
                    sin5 = sn.unsqueeze(2).broadcast_to([P, g1_ - g0, 5, H32])
                    q1 = qv5[:, :, :, 0:H32]
                    q2 = qv5[:, :, :, H32:D]
                    nc.vector.tensor_mul(ro5[:, :, :, 0:H32], q1, cos5)
                    nc.vector.tensor_mul(t5[:], q2, sin5)
                    nc.vector.tensor_add(
                        ro5[:, :, :, 0:H32], ro5[:, :, :, 0:H32], t5[:]
                    )
                    nc.vector.tensor_mul(ro5[:, :, :, H32:D], q2, cos5)
                    nc.vector.tensor_mul(t5[:], q1, sin5)
                    nc.vector.tensor_sub(
                        ro5[:, :, :, H32:D], ro5[:, :, :, H32:D], t5[:]
                    )

                    nc.vector.tensor_mul(
                        sqg[:, g0:g1_, :], qkr[:, g0:g1_, :], qkr[:, g0:g1_, :]
                    )
                    nc.vector.reduce_sum(
                        msg[:, f0:f1],
                        sqg[:, g0:g1_, :].rearrange("p g (h d) -> p (g h) d", d=D),
                        axis=mybir.AxisListType.X,
                    )
                    # m = mean + eps; rstd = m^-1/2 by bit-trick seed + two
                    # Newton iterations, entirely on DVE (no ACT Ln table).
                    nc.vector.tensor_scalar(
                        msg[:, f0:f1], msg[:, f0:f1], 1.0 / D, 1e-6,
                        op0=mybir.AluOpType.mult, op1=mybir.AluOpType.add,
                    )
                    rstdu = rstdg[:, f0:f1].bitcast(mybir.dt.uint32)
                    nc.vector.tensor_scalar(
                        rstdu, msg[:, f0:f1].bitcast(mybir.dt.uint32), 1, None,
                        op0=mybir.AluOpType.logical_shift_right,
                    )
                    nc.vector.tensor_sub(
                        rstdu,
                        rsq_k[:].broadcast_to([P, nf]).bitcast(mybir.dt.uint32),
                        rstdu,
                    )
                    # one Newton step suffices: 3.4% seed error -> ~0.2%,
                    # well inside the bf16 data path's noise floor
                    for _ in range(1):
                        nc.vector.tensor_mul(
                            nwt[:, f0:f1], msg[:, f0:f1], rstdg[:, f0:f1]
                        )
                        nc.vector.tensor_mul(
                            nwt[:, f0:f1], nwt[:, f0:f1], rstdg[:, f0:f1]
                        )
                        nc.vector.tensor_scalar(
                            nwt[:, f0:f1], nwt[:, f0:f1], -0.5, 1.5,
                            op0=mybir.AluOpType.mult, op1=mybir.AluOpType.add,
                        )
                        nc.vector.tensor_mul(
                            rstdg[:, f0:f1], rstdg[:, f0:f1], nwt[:, f0:f1]
                        )
                    # normalize per token chunk so each chunk's transposes
                    # unblock as soon as its multiply lands
                    for g in range(g0, g1_):
                        nc.vector.tensor_mul(
                            qkn[:, g, :].rearrange("p (h d) -> p h d", d=D),
                            qkr[:, g, :].rearrange("p (h d) -> p h d", d=D),
                            rstdg[:, 5 * g : 5 * (g + 1)]
                            .unsqueeze(2)
                            .broadcast_to([P, 5, D]),
                        )
                # gate r = sigmoid(z) = 0.5 + 0.5*tanh(z/2); ve3 is 3*ve.
                # Emitted after the rms chain so the DVE reaches the chain
                # sooner; elementwise v work runs on the idle GPSIMD.
                rgg = small.tile([P, GRP], f32, tag="rgg")
                nc.vector.tensor_scalar(
                    rgg[:], tgg[:], 0.5, 0.5,
                    op0=mybir.AluOpType.mult, op1=mybir.AluOpType.add,
                )
                vtg = small.tile([P, GRP, D], f32, tag="vtg", bufs=1)
                nc.gpsimd.tensor_mul(
                    vtg[:],
                    ve3_sb[:, bi * GRP : (bi + 1) * GRP, :],
                    rgg[:].unsqueeze(2).broadcast_to([P, GRP, D]),
                )
                nc.gpsimd.tensor_add(
                    v_aug[:, bi * GRP : (bi + 1) * GRP, 0:D],
                    pjg[:, :, 320:384],
                    vtg[:],
                )
                qkns[bi] = qkn

            def phase1b(bi):
                qkn = qkns.pop(bi)
                # transposes (two heads per [128,128] transpose)
                tpk = pssc.tile([D, 512], bf16, tag="sc")
                for tl in range(GRP):
                    tc_ = bi * GRP + tl
                    tp = pssc.tile([P, 256], bf16, tag="sc")
                    nc.tensor.transpose(
                        tp[:, 0:P], qkn[:, tl, 0:128], identb[:]
                    )
                    nc.tensor.transpose(
                        tp[:, P : 2 * P], qkn[:, tl, 128:256], identb[:]
                    )
                    nc.tensor.transpose(
                        tpk[:, tl * P : (tl + 1) * P], qkn[:, tl, 256:320], identb[:]
                    )
                    nc.vector.tensor_copy(
                        qT[:, :, tc_ * P : (tc_ + 1) * P],
                        tp[:].rearrange("p (g t) -> p g t", g=2),
                    )
                nc.vector.tensor_copy(kT2[0:D, bi * IB : (bi + 1) * IB], tpk[:])
                nc.vector.tensor_copy(kT2[D:P, bi * IB : (bi + 1) * IB], tpk[:])

            def phase2(bi, after_head=None):
                yns = []
                for h in range(HQ):
                    # y accumulated [query, qc, d | den]: one PSUM bank, four
                    # per-qc accumulation regions.  Only (jt=0, qc=0) starts;
                    # the bank-wide pending-zero makes the other qc's first
                    # write a plain store (skip_group_check for the regions).
                    yp = psy.tile([P, GRP, D + 1], f32, tag="y")
                    njt = GRP * (bi + 1)
                    rr = D * (h % 2)
                    qTh = qT[rr : rr + D, h // 2, :]

                    nfull = GRP * bi + 1  # tiles with lo == 0
                    pending = []

                    def score_mm(spc, jt):
                        dg = jt - GRP * bi
                        lo = max(dg, 0) * P
                        nc.tensor.matmul(
                            spc[:, lo:512],
                            kT2[rr : rr + D, jt * P : (jt + 1) * P],
                            qTh[:, bi * IB + lo : (bi + 1) * IB],
                            start=True,
                            stop=(dg < 0),
                        )
                        if dg >= 0:
                            # additive causal mask: -3e4 above the diagonal so
                            # exp underflows to exactly 0 (no Pool multiply)
                            nc.tensor.matmul(
                                spc[:, lo : lo + P],
                                trin_sb[:],
                                identb[:],
                                start=False,
                                stop=True,
                            )
                        return lo, dg

                    def emit_av(jt, exap, dg, qoff):
                        # stationary = exp scores [128 keys, 128 queries],
                        # moving = v_aug [128 keys, 65]: 65-col streams (the
                        # ldweights swap is free) instead of 512-col streams;
                        # exap col 0 corresponds to query qoff*128
                        for qc in range(GRP):
                            if dg > qc:
                                continue  # whole qc block above the diagonal
                            nc.tensor.matmul(
                                yp[:, qc, :],
                                exap[:, (qc - qoff) * P : (qc - qoff + 1) * P],
                                v_aug[:, jt, :],
                                start=(jt == 0 and qc == 0),
                                stop=(jt == GRP * bi + qc),
                                skip_group_check=True,
                            )

                    def flush(n):
                        while len(pending) > n:
                            emit_av(*pending.pop(0))

                    jt = 0
                    while jt < njt:
                        dg = jt - GRP * bi
                        if dg == 2:
                            # the two smallest diagonal partials (widths
                            # 256+128) share one PSUM bank and ONE exp,
                            # saving the 185ns fixed ACT cost per pair
                            sp = pssc.tile([P, 384], f32, tag="sc", name="sp")
                            ex = exps.tile([P, 384], bf16, tag="ex", name="ex")
                            nc.tensor.matmul(
                                sp[:, 0:256],
                                kT2[rr : rr + D, jt * P : (jt + 1) * P],
                                qTh[:, bi * IB + 256 : (bi + 1) * IB],
                                start=True, stop=False,
                            )
                            nc.tensor.matmul(
                                sp[:, 0:P], trin_sb[:], identb[:],
                                start=False, stop=True,
                            )
                            nc.tensor.matmul(
                                sp[:, 256:384],
                                kT2[rr : rr + D, (jt + 1) * P : (jt + 2) * P],
                                qTh[:, bi * IB + 384 : (bi + 1) * IB],
                                start=True, stop=False,
                            )
                            nc.tensor.matmul(
                                sp[:, 256:384], trin_sb[:], identb[:],
                                start=False, stop=True,
                            )
                            nc.scalar.activation(
                                ex[:], sp[:], AF.Exp, scale=SC, bias=zero_sb[:],
                            )
                            pending.append((jt, ex[:, 0:256], 2, 2))
                            pending.append((jt + 1, ex[:, 256:384], 3, 3))
                            jt += 2
                        else:
                            sp = pssc.tile([P, 512], f32, tag="sc", name="sp")
                            ex = exps.tile([P, 512], bf16, tag="ex", name="ex")
                            lo, dg = score_mm(sp, jt)
                            nc.scalar.activation(
                                ex[:, lo:512], sp[:, lo:512], AF.Exp,
                                scale=SC, bias=zero_sb[:],
                            )
                            pending.append((jt, ex, dg, 0))
                            jt += 1
                        flush(3)
                    flush(0)
                    # the last head's hook (next group's phase1b) fires
                    # before its normalize tail so the qT/kT2 copies get
                    # ahead of the tail ops in the in-order DVE queue: the
                    # next block's first scores depend on them
                    if after_head is not None:
                        after_head(h)
                        fired_last = True
                    else:
                        fired_last = False
                    # normalize in [query, d] orientation: per-partition
                    # denominator scalars.  The PE transposes into the yT
                    # residents are deferred to the end of the bi block so the
                    # in-order PE queue never parks on a transpose whose yn
                    # input is still deep in the DVE queue (that would starve
                    # the score stream and the exp pipeline behind it).
                    rec = small.tile([P, GRP], f32, tag="rec")
                    nc.vector.reciprocal_approx_fast(rec[:], yp[:, :, D])
                    # hybrid normalize: one DVE bulk copy out of PSUM, then
                    # the per-qc scalar multiplies on the idle Pool engine
                    yn = ynp.tile([P, GRP, D], bf16, tag="yn")
                    if bi == NBI - 1 and h == HQ - 1:
                        # drain tail: direct DVE normalize, shortest chain
                        for qc in range(GRP):
                            nc.vector.tensor_scalar(
                                yn[:, qc, :], yp[:, qc, 0:D], rec[:, qc : qc + 1],
                                None, op0=mybir.AluOpType.mult,
                            )
                    else:
                        # hybrid: one DVE bulk copy out of PSUM, then the
                        # per-qc scalar multiplies on the idle Pool engine
                        yc = ynp.tile([P, GRP, D], bf16, tag="yc")
                        nc.vector.tensor_copy(yc[:], yp[:, :, 0:D])
                        for qc in range(GRP):
                            nc.gpsimd.tensor_scalar(
                                yn[:, qc, :], yc[:, qc, :], rec[:, qc : qc + 1],
                                None, op0=mybir.AluOpType.mult,
                            )
                    yns.append((h, yn))
                    if after_head is not None and not fired_last:
                        after_head(h)
                for h, yn in yns:
                    stg = psy.tile([D, IB], bf16, tag="y")
                    for qc in range(GRP):
                        nc.tensor.transpose(
                            stg[:, qc * P : (qc + 1) * P], yn[:, qc, :], identb[:]
                        )
                    ytp = yT1 if h < 2 else yT2
                    row = D * (h % 2)
                    nc.vector.tensor_copy(
                        ytp[row : row + D, bi * IB : (bi + 1) * IB], stg[:]
                    )

            def norm3w(bi, split_copies=False):
                # in the drain tail the score pool is free: 4 po slots keep
                # the Wo stream, readout copies and out-DMAs fully pipelined
                for tl in range(GRP):
                    tc_ = bi * GRP + tl
                    ob = outsb.tile([P, C], bf16, tag="ob")
                    for cb in range(2):
                        if split_copies:
                            po = pssc.tile([P, 512], f32, tag="sc")
                        else:
                            po = psmm.tile([P, 512], f32, tag="mm")
                        nc.tensor.matmul(
                            po[:],
                            yT1[:, tc_ * P : (tc_ + 1) * P],
                            wo1_sb[:, cb * 512 : (cb + 1) * 512],
                            start=True,
                            stop=False,
                        )
                        nc.tensor.matmul(
                            po[:],
                            yT2[:, tc_ * P : (tc_ + 1) * P],
                            wo2_sb[:, cb * 512 : (cb + 1) * 512],
                            start=False,
                            stop=True,
                        )
                        # in the drain tail ACT is idle: alternate the PSUM
                        # readout between DVE and ACT so po slots recycle 2x
                        # faster
                        if split_copies and cb == 1:
                            nc.scalar.copy(ob[:, cb * 512 : (cb + 1) * 512], po[:])
                        else:
                            nc.vector.tensor_copy(
                                ob[:, cb * 512 : (cb + 1) * 512], po[:]
                            )
                    nc.sync.dma_start(out[tc_ * P : (tc_ + 1) * P, :], ob[:])

            # group-level software pipeline: next group's projections are
            # emitted before the previous group's Wo so the PE has ready work
            # while the per-head normalize chains resolve.
            xts = {0: xt0, 1: xt1}
            qkns = {}
            phase1(0, xts[0], halves=2)
            phase1b(0)

            def hook0(h):
                # group 1's projections split across the thin bi-0 heads so
                # the 5us proj block doesn't starve the exp stream
                if h == 0:
                    phase1_proj(1, xts[1], [0, 1])
                    xts[2] = load_x(2)
                elif h == 1:
                    phase1_proj(1, xts[1], [2, 3])
                elif h == 2:
                    phase1(1, xts[1], halves=2, proj=False)
                elif h == 3:
                    phase1b(1)

            phase2(0, after_head=hook0)
            for bi in range(1, NBI):
                # the next group's projections, transposes and bi-1's
                # Wo/writeout are deferred into phase2 via the per-head hook
                # so they do not sit ahead of the score/exp stream in the
                # in-order queues, and so the transposes finish well before
                # the group boundary
                def hook(h, bi=bi):
                    if h == 0:
                        if bi + 1 < NBI:
                            if bi == 1:
                                # phase2(1) is still thin: split group 2's
                                # projection block across two head hooks
                                phase1_proj(2, xts[2], [0, 1])
                            else:
                                phase1(bi + 1, xts[bi + 1])
                            if bi + 2 < NBI:
                                xts[bi + 2] = load_x(bi + 2)
                    elif h == 1:
                        if bi == 1:
                            phase1_proj(2, xts[2], [2, 3])
                            phase1(2, xts[2], proj=False)
                        norm3w(bi - 1)
                    elif h == 3 and bi + 1 < NBI:
                        phase1b(bi + 1)

                phase2(bi, after_head=hook)
                if bi == NBI - 1:
                    norm3w(bi, split_copies=True)
    nc.compile()
    return nc


def make_core_inputs(x, ve, cos, sin, Wq, Wk, Wv, Wo, Wg):
    """Slice full inputs into the 8 per-core input maps (b-major, then group)."""
    import ml_dtypes

    bf = ml_dtypes.bfloat16
    # device layout [P, NT*32]: row p holds cos[n*128+p, :] for n in 0..NT
    cosf = np.ascontiguousarray(
        cos[0, :, 0, :].reshape(NT, P, 32).transpose(1, 0, 2).reshape(P, NT * 32)
    ).astype(bf)
    sinf = np.ascontiguousarray(
        sin[0, :, 0, :].reshape(NT, P, 32).transpose(1, 0, 2).reshape(P, NT * 32)
    ).astype(bf)
    # trin[c, k] = -3e4 where key k > query c (strict upper): additive mask
    # accumulated into the diagonal score band via trin^T (identity moving).
    trin = np.where(
        np.arange(P)[None, :] > np.arange(P)[:, None], -30000.0, 0.0
    ).astype(bf)
    in_maps = []
    for c in range(8):
        b, g = c // N_KV_HEAD, c % N_KV_HEAD
        xTc = np.ascontiguousarray(x[b].T).astype(bf)  # [C, T]
        wq = Wq[g * 256 : (g + 1) * 256, :]           # [256, C]
        wk = Wk[g * D : (g + 1) * D, :]               # [64, C]
        wv = Wv[g * D : (g + 1) * D, :]
        wg_col = np.zeros((C, 1), np.float32)
        wg_col[:12, 0] = Wg[g]
        wrc = np.concatenate(
            [wq.T, wk.T, wv.T, wg_col, np.zeros((C, 1), np.float32)], axis=1
        ).astype(bf)                                  # [C, 386]
        ve3 = np.ascontiguousarray(
            (3.0 * ve[b, :, g * D : (g + 1) * D])
            .reshape(NT, P, D).transpose(1, 0, 2).reshape(P, NT * D)
        ).astype(bf)                                  # [P, NT*64]
        woTc = np.ascontiguousarray(
            Wo[:, g * 256 : (g + 1) * 256].T
        ).astype(bf)                                  # [256, C]
        in_maps.append(
            {
                "xT": xTc,
                "wr": np.ascontiguousarray(wrc),
                "cosd": cosf,
                "sind": sinf,
                "ve3": ve3,
                "woT": woTc,
                "trind": trin,
            }
        )
    return in_maps


_PROGRAM = None


def kernel(x, ve, cos, sin, Wq, Wk, Wv, Wo, Wg, _trace=False):
    from concourse.bass_utils import run_bass_kernel_spmd

    # coerce to host fp32 ndarrays up front (harness may pass jax arrays)
    x, ve, cos, sin, Wq, Wk, Wv, Wo, Wg = (
        np.asarray(a, dtype=np.float32)
        for a in (x, ve, cos, sin, Wq, Wk, Wv, Wo, Wg)
    )
    global _PROGRAM
    if _PROGRAM is None:
        _PROGRAM = build_program()
    nc = _PROGRAM
    in_maps = make_core_inputs(x, ve, cos, sin, Wq, Wk, Wv, Wo, Wg)
    res = run_bass_kernel_spmd(nc, in_maps, list(range(8)), trace=_trace)
    outs = [r["out"] for r in res.results]
    full = np.zeros((B, T, C), np.float32)
    for c in range(8):
        full[c // N_KV_HEAD] += np.asarray(outs[c], dtype=np.float32)
    if _trace:
        kernel.last_results = res
    return full



# revision 85
# speedup vs baseline: 1.0328x; 1.0040x over previous
"""Trainium2 Bass kernel for nn_Attention_5299989643989.

GQA attention forward (B=2, T=2048, C=1024, 16 q heads / 4 kv heads, D=64)
with value-embedding gating, rotary embedding, qk rms-norm, causal softmax.

Sharding: 8 cores = batch (2) x kv-head-group (4).  Each core computes its
4 q heads / 1 kv head end-to-end plus the Wo row-shard partial output; the
host sums the 4 partials per batch (the Wo all-reduce, done at unshard).

Per-core structure (bf16 data paths, fp32 PSUM accumulate; ~141us/core in
the TRN2 cost-model timeline at rel err 5.7e-3, vs 175us/2.6e-4 for the
all-fp32r ancestor). Emission order is tuned so the score/exp stream owns
the in-order engine-queue heads: the next group's projections and the
previous group's normalize/Wo are emitted INSIDE phase2 via a per-head
hook (deferral placement is performance-sensitive).
  phase1a: per 128-token chunk one jammed projection matmul
           [q(256)|k(64)|v(64)|gate(1)|pad] over bf16 x/W accumulated in
           PSUM and copied to a bf16 SBUF group tile; per 4-chunk group:
           rope over all 20 head instances via 4D strided bf16 views (2x
           DVE), rms rstd via bit-trick + ONE Newton step on DVE (bf16
           squares), per-chunk normalize multiplies so each chunk's
           transposes unblock early, one batched sigmoid-gate Tanh,
           ve-gating on GPSIMD.
  phase1b: paired 2-head bf16 PE transposes (1 cycle/row) into qT
           [128,2,T] and row-duplicated kT2 [128,T] (matmul requires equal
           stationary/moving partition bases).
  phase2:  per (head, 512-query block): scoresT tiles [128 keys, <=512
           live queries] = kT^T q; the causal mask is a -3e4 additive
           mask-matmul (trin^T x identity_bf16) accumulated into the
           diagonal PSUM band so exp underflows masked lanes to exactly 0
           (no Pool multiply, no extra engine hop); exp on ACT (the one
           saturated engine mid-run) with the folded 1.2*1.2/sqrt(64)
           scale, bf16 out; yT [65,512] += v_aug^T expT with a ones column
           producing denominators for free; 4-slot PSUM score pipeline
           with depth-4 AV stagger, 9 ex buffers.
  norm3:   denominator reciprocals on DVE, f32r rounding copy (Pool
           mid-run / ACT in the drain tail), PE outer-product broadcast,
           yT scaling, row-sharded bf16 Wo, per-chunk [128,1024] staging
           tile with PSUM readouts alternating DVE/ACT in the tail, one
           DMA per token chunk.
  Software pipeline at emission: group bi+1's projections and DVE chain
  are emitted before phase2(bi); normalize/Wo of bi-1 fill the PE while
  bi's transposes wait on the DVE chain; the last group's pair-0
  normalize is emitted mid-phase2 so only pair 1 + Wo remain in the tail.
Host side ships bf16 inputs (x^T, W-jam, cos/sin, 3*ve, Wo^T, mask) and
upcasts the bf16 per-core partials while summing the Wo row-shards.
"""

import numpy as np

import concourse.bacc as bacc
import concourse.bass as bass
import concourse.tile as tile
from concourse import mybir
from concourse.masks import make_identity

f32 = mybir.dt.float32
f32r = mybir.dt.float32r
bf16 = mybir.dt.bfloat16
AF = mybir.ActivationFunctionType

B, T, C = 2, 2048, 1024
N_HEAD, N_KV_HEAD, D = 16, 4, 64
HQ = N_HEAD // N_KV_HEAD  # q heads per core = 4
P = 128
NT = T // P       # 16 token chunks
KC = C // P       # 8 contraction chunks
IB = 512          # query block
NBI = T // IB     # 4 query blocks
GRP = IB // P     # 4 token chunks per query block
SC = 1.2 * 1.2 / 8.0  # folded qk scale: rms 1.2 factors * 1/sqrt(64)
H32 = D // 2


def build_program():
    nc = bacc.Bacc("TRN2", target_bir_lowering=False, debug=False, num_devices=8)

    xT = nc.dram_tensor("xT", [C, T], bf16, kind="ExternalInput")
    wr = nc.dram_tensor("wr", [C, 386], bf16, kind="ExternalInput")
    # cos/sin/ve3 pre-swizzled on the host into the SBUF-resident layout so
    # the DMA moves long contiguous rows (512B+ descriptors, full bandwidth)
    cosd = nc.dram_tensor("cosd", [P, NT * 32], bf16, kind="ExternalInput")
    sind = nc.dram_tensor("sind", [P, NT * 32], bf16, kind="ExternalInput")
    ve3 = nc.dram_tensor("ve3", [P, NT * D], bf16, kind="ExternalInput")
    woT = nc.dram_tensor("woT", [2 * P, C], bf16, kind="ExternalInput")
    trind = nc.dram_tensor("trind", [P, P], bf16, kind="ExternalInput")
    out = nc.dram_tensor("out", [T, C], bf16, kind="ExternalOutput")

    with tile.TileContext(nc) as tc:
        with (
            tc.tile_pool(name="consts", bufs=1) as consts,
            tc.tile_pool(name="resid", bufs=1) as resid,
            tc.tile_pool(name="xload", bufs=2) as xload,
            tc.tile_pool(name="rot", bufs=2) as rot,
            tc.tile_pool(name="small", bufs=6) as small,
            tc.tile_pool(name="exps", bufs=9) as exps,
            tc.tile_pool(name="ynp", bufs=5) as ynp,
            tc.tile_pool(name="outsb", bufs=4) as outsb,
            tc.tile_pool(name="psmm", bufs=2, space="PSUM") as psmm,
            tc.tile_pool(name="pssc", bufs=4, space="PSUM") as pssc,
            tc.tile_pool(name="psy", bufs=2, space="PSUM") as psy,
        ):
            # ---- resident loads ----
            # The cost-model DMA lane is serial (~0.003 ns/B), so order
            # strictly by need: weights + cos/sin first (small), then x block
            # 0 token-major in 4 pieces so each projection chunk can run as
            # its tokens arrive, then everything else.
            wr_sb = consts.tile([P, KC, 386], bf16)
            nc.sync.dma_start(
                wr_sb[:, 0, :],
                wr[0:P, :],
            )
            nc.sync.dma_start(
                wr_sb[:, 1:KC, :],
                wr[P:C, :].rearrange("(kc p) c -> p kc c", p=P),
            )
            xt0 = xload.tile([P, KC, IB], bf16, name="xt0", tag="xt")
            nc.sync.dma_start(
                xt0[:, 0 : KC // 2, :],
                xT[0 : C // 2, 0:IB].rearrange("(kc p) t -> p kc t", p=P),
            )
            nc.sync.dma_start(
                xt0[:, KC // 2 : KC, :],
                xT[C // 2 : C, 0:IB].rearrange("(kc p) t -> p kc t", p=P),
            )
            cos_sb = consts.tile([P, NT, 32], bf16)
            nc.sync.dma_start(cos_sb[:].rearrange("p n d -> p (n d)"), cosd[:])
            sin_sb = consts.tile([P, NT, 32], bf16)
            nc.sync.dma_start(sin_sb[:].rearrange("p n d -> p (n d)"), sind[:])
            xt1 = xload.tile([P, KC, IB], bf16, name="xt1", tag="xt")
            nc.sync.dma_start(
                xt1[:],
                xT[:, IB : 2 * IB].rearrange("(kc p) t -> p kc t", p=P),
            )
            ve3_sb = consts.tile([P, NT, D], bf16)
            nc.sync.dma_start(ve3_sb[:].rearrange("p n d -> p (n d)"), ve3[:])
            trin_sb = consts.tile([P, P], bf16)
            nc.sync.dma_start(trin_sb[:], trind[:])
            wo1_sb = consts.tile([P, C], bf16)
            nc.sync.dma_start(wo1_sb[:], woT[0:P, :])
            wo2_sb = consts.tile([P, C], bf16)
            nc.sync.dma_start(wo2_sb[:], woT[P : 2 * P, :])
            ident = consts.tile([P, P], f32)
            make_identity(nc, ident[:])
            identb = consts.tile([P, P], bf16)
            nc.vector.tensor_copy(identb[:], ident[:])
            # PE p-state warmup: keep the PE streaming through the initial
            # DMA wait so the first projection matmuls run at full clock
            # (cold PE costs 1.5-3.7x per matmul until 3us of busy ramp)
            warm = psy.tile([P, P], f32, tag="y")
            for _ in range(44):
                nc.tensor.matmul(warm[:], identb[:], identb[:])
            rsq_k = consts.tile([P, 1], mybir.dt.uint32)
            nc.vector.memset(rsq_k[:], 0x5F3759DF)
            zero_sb = consts.tile([P, 1], f32)
            nc.vector.memset(zero_sb[:], 0.0)

            # ---- residents written by the kernel ----
            qT = resid.tile([P, 2, T], bf16)   # [h0|h1] rows, [h2|h3] rows
            kT2 = resid.tile([P, T], bf16)     # kT duplicated in both row halves
            v_aug = resid.tile([P, NT, D + 1], bf16)  # v plus ones column
            nc.vector.memset(v_aug[:, :, D : D + 1], 1.0)
            yT1 = resid.tile([P, T], bf16)        # yT heads 0,1
            yT2 = resid.tile([P, T], bf16)        # yT heads 2,3

            def load_x(bi, q=None):
                xt = xload.tile([P, KC, IB], bf16, name=f"xt{bi}", tag="xt")
                (q or nc.gpsimd).dma_start(
                    xt[:],
                    xT[:, bi * IB : (bi + 1) * IB]
                    .rearrange("(kc p) t -> p kc t", p=P),
                )
                return xt

            pjgs = {}

            def phase1_proj(bi, xt, tls):
                if bi not in pjgs:
                    pjgs[bi] = rot.tile([P, GRP, 386], bf16, tag="pjg", name=f"pjg{bi}")
                pjg = pjgs[bi]
                for tl in tls:
                    tc_ = bi * GRP + tl
                    pj = psmm.tile([P, 512], f32, tag="mm")
                    for kc in range(KC):
                        nc.tensor.matmul(
                            pj[:, 0:386],
                            xt[:, kc, tl * P : (tl + 1) * P],
                            wr_sb[:, kc, :],
                            start=(kc == 0),
                            stop=(kc == KC - 1),
                        )
                    # ACT for the startup groups (latency-critical chain);
                    # DVE for the mid-run groups where ACT is exp-saturated
                    if bi < 2:
                        nc.scalar.copy(pjg[:, tl, :], pj[:, 0:386])
                    else:
                        nc.vector.tensor_copy(pjg[:, tl, :], pj[:, 0:386])

            def phase1(bi, xt, halves=1, proj=True):
                if proj:
                    phase1_proj(bi, xt, range(GRP))
                qkr = rot.tile([P, GRP, 320], bf16, tag="qkr", bufs=1)  # roped q|k
                pjg = pjgs.pop(bi)
                tgg = small.tile([P, GRP], f32, tag="tgg")
                # one tanh over all 4 chunks' gate column (already staged in
                # the pjg copy) instead of 4 per-chunk PSUM reads
                nc.scalar.activation(
                    tgg[:], pjg[:, :, 384], AF.Tanh,
                    scale=0.5, bias=zero_sb[:],
                )

                # rope + rms + rstd + normalize over the group, optionally in
                # two chunk-pair halves (shorter DVE chain before the
                # transposes, at the cost of ~16 extra small DVE ops).
                tmp = rot.tile([P, GRP, 160], bf16, tag="tmp", bufs=1)
                sqg = rot.tile([P, GRP, 320], bf16, tag="sqg", bufs=1)
                msg = small.tile([P, GRP * 5], f32, tag="msg")
                rstdg = small.tile([P, GRP * 5], f32, tag="rstdg")
                nwt = small.tile([P, GRP * 5], f32, tag="nwt")
                qkn = rot.tile([P, GRP, 320], bf16, tag="qkn", bufs=2)
                gstep = GRP // halves
                for hf in range(halves):
                    g0, g1_ = gstep * hf, gstep * (hf + 1)
                    f0, f1 = 5 * gstep * hf, 5 * gstep * (hf + 1)
                    nf = f1 - f0
                    qv5 = pjg[:, g0:g1_, 0:320].rearrange(
                        "p g (h d) -> p g h d", d=D
                    )
                    ro5 = qkr[:, g0:g1_, :].rearrange("p g (h d) -> p g h d", d=D)
                    t5 = tmp[:, g0:g1_, :].rearrange("p g (h d) -> p g h d", d=H32)
                    cs = cos_sb[:, bi * GRP + g0 : bi * GRP + g1_, :]
                    sn = sin_sb[:, bi * GRP + g0 : bi * GRP + g1_, :]
                    cos5 = cs.unsqueeze(2).broadcast_to([P, g1_ - g0, 5, H32])
                    sin5 = sn.unsqueeze(2).broadcast_to([P, g1_ - g0, 5, H32])
                    q1 = qv5[:, :, :, 0:H32]
                    q2 = qv5[:, :, :, H32:D]
                    nc.vector.tensor_mul(ro5[:, :, :, 0:H32], q1, cos5)
                    nc.vector.tensor_mul(t5[:], q2, sin5)
                    nc.vector.tensor_add(
                        ro5[:, :, :, 0:H32], ro5[:, :, :, 0:H32], t5[:]
                    )
                    nc.vector.tensor_mul(ro5[:, :, :, H32:D], q2, cos5)
                    nc.vector.tensor_mul(t5[:], q1, sin5)
                    nc.vector.tensor_sub(
                        ro5[:, :, :, H32:D], ro5[:, :, :, H32:D], t5[:]
                    )

                    nc.vector.tensor_mul(
                        sqg[:, g0:g1_, :], qkr[:, g0:g1_, :], qkr[:, g0:g1_, :]
                    )
                    nc.vector.reduce_sum(
                        msg[:, f0:f1],
                        sqg[:, g0:g1_, :].rearrange("p g (h d) -> p (g h) d", d=D),
                        axis=mybir.AxisListType.X,
                    )
                    # m = mean + eps; rstd = m^-1/2 by bit-trick seed + two
                    # Newton iterations, entirely on DVE (no ACT Ln table).
                    nc.vector.tensor_scalar(
                        msg[:, f0:f1], msg[:, f0:f1], 1.0 / D, 1e-6,
                        op0=mybir.AluOpType.mult, op1=mybir.AluOpType.add,
                    )
                    rstdu = rstdg[:, f0:f1].bitcast(mybir.dt.uint32)
                    nc.vector.tensor_scalar(
                        rstdu, msg[:, f0:f1].bitcast(mybir.dt.uint32), 1, None,
                        op0=mybir.AluOpType.logical_shift_right,
                    )
                    nc.vector.tensor_sub(
                        rstdu,
                        rsq_k[:].broadcast_to([P, nf]).bitcast(mybir.dt.uint32),
                        rstdu,
                    )
                    # one Newton step suffices: 3.4% seed error -> ~0.2%,
                    # well inside the bf16 data path's noise floor
                    for _ in range(1):
                        nc.vector.tensor_mul(
                            nwt[:, f0:f1], msg[:, f0:f1], rstdg[:, f0:f1]
                        )
                        nc.vector.tensor_mul(
                            nwt[:, f0:f1], nwt[:, f0:f1], rstdg[:, f0:f1]
                        )
                        nc.vector.tensor_scalar(
                            nwt[:, f0:f1], nwt[:, f0:f1], -0.5, 1.5,
                            op0=mybir.AluOpType.mult, op1=mybir.AluOpType.add,
                        )
                        nc.vector.tensor_mul(
                            rstdg[:, f0:f1], rstdg[:, f0:f1], nwt[:, f0:f1]
                        )
                    # normalize per token chunk so each chunk's transposes
                    # unblock as soon as its multiply lands
                    for g in range(g0, g1_):
                        nc.vector.tensor_mul(
                            qkn[:, g, :].rearrange("p (h d) -> p h d", d=D),
                            qkr[:, g, :].rearrange("p (h d) -> p h d", d=D),
                            rstdg[:, 5 * g : 5 * (g + 1)]
                            .unsqueeze(2)
                            .broadcast_to([P, 5, D]),
                        )
                # gate r = sigmoid(z) = 0.5 + 0.5*tanh(z/2); ve3 is 3*ve.
                # Emitted after the rms chain so the DVE reaches the chain
                # sooner; elementwise v work runs on the idle GPSIMD.
                rgg = small.tile([P, GRP], f32, tag="rgg")
                nc.vector.tensor_scalar(
                    rgg[:], tgg[:], 0.5, 0.5,
                    op0=mybir.AluOpType.mult, op1=mybir.AluOpType.add,
                )
                vtg = small.tile([P, GRP, D], f32, tag="vtg", bufs=1)
                nc.gpsimd.tensor_mul(
                    vtg[:],
                    ve3_sb[:, bi * GRP : (bi + 1) * GRP, :],
                    rgg[:].unsqueeze(2).broadcast_to([P, GRP, D]),
                )
                nc.gpsimd.tensor_add(
                    v_aug[:, bi * GRP : (bi + 1) * GRP, 0:D],
                    pjg[:, :, 320:384],
                    vtg[:],
                )
                qkns[bi] = qkn

            def phase1b(bi):
                qkn = qkns.pop(bi)
                # transposes (two heads per [128,128] transpose)
                tpk = pssc.tile([D, 512], bf16, tag="sc")
                for tl in range(GRP):
                    tc_ = bi * GRP + tl
                    tp = pssc.tile([P, 256], bf16, tag="sc")
                    nc.tensor.transpose(
                        tp[:, 0:P], qkn[:, tl, 0:128], identb[:]
                    )
                    nc.tensor.transpose(
                        tp[:, P : 2 * P], qkn[:, tl, 128:256], identb[:]
                    )
                    nc.tensor.transpose(
                        tpk[:, tl * P : (tl + 1) * P], qkn[:, tl, 256:320], identb[:]
                    )
                    nc.vector.tensor_copy(
                        qT[:, :, tc_ * P : (tc_ + 1) * P],
                        tp[:].rearrange("p (g t) -> p g t", g=2),
                    )
                nc.vector.tensor_copy(kT2[0:D, bi * IB : (bi + 1) * IB], tpk[:])
                nc.vector.tensor_copy(kT2[D:P, bi * IB : (bi + 1) * IB], tpk[:])

            def phase2(bi, after_head=None):
                yns = []
                for h in range(HQ):
                    # y accumulated [query, qc, d | den]: one PSUM bank, four
                    # per-qc accumulation regions.  Only (jt=0, qc=0) starts;
                    # the bank-wide pending-zero makes the other qc's first
                    # write a plain store (skip_group_check for the regions).
                    yp = psy.tile([P, GRP, D + 1], f32, tag="y")
                    njt = GRP * (bi + 1)
                    rr = D * (h % 2)
                    qTh = qT[rr : rr + D, h // 2, :]

                    nfull = GRP * bi + 1  # tiles with lo == 0
                    pending = []

                    def score_mm(spc, jt):
                        dg = jt - GRP * bi
                        lo = max(dg, 0) * P
                        nc.tensor.matmul(
                            spc[:, lo:512],
                            kT2[rr : rr + D, jt * P : (jt + 1) * P],
                            qTh[:, bi * IB + lo : (bi + 1) * IB],
                            start=True,
                            stop=(dg < 0),
                        )
                        if dg >= 0:
                            # additive causal mask: -3e4 above the diagonal so
                            # exp underflows to exactly 0 (no Pool multiply)
                            nc.tensor.matmul(
                                spc[:, lo : lo + P],
                                trin_sb[:],
                                identb[:],
                                start=False,
                                stop=True,
                            )
                        return lo, dg

                    def emit_av(jt, exap, dg, qoff):
                        # stationary = exp scores [128 keys, 128 queries],
                        # moving = v_aug [128 keys, 65]: 65-col streams (the
                        # ldweights swap is free) instead of 512-col streams;
                        # exap col 0 corresponds to query qoff*128
                        for qc in range(GRP):
                            if dg > qc:
                                continue  # whole qc block above the diagonal
                            nc.tensor.matmul(
                                yp[:, qc, :],
                                exap[:, (qc - qoff) * P : (qc - qoff + 1) * P],
                                v_aug[:, jt, :],
                                start=(jt == 0 and qc == 0),
                                stop=(jt == GRP * bi + qc),
                                skip_group_check=True,
                            )

                    def flush(n):
                        while len(pending) > n:
                            emit_av(*pending.pop(0))

                    jt = 0
                    while jt < njt:
                        dg = jt - GRP * bi
                        if dg == 2:
                            # the two smallest diagonal partials (widths
                            # 256+128) share one PSUM bank and ONE exp,
                            # saving the 185ns fixed ACT cost per pair
                            sp = pssc.tile([P, 384], f32, tag="sc", name="sp")
                            ex = exps.tile([P, 384], bf16, tag="ex", name="ex")
                            nc.tensor.matmul(
                                sp[:, 0:256],
                                kT2[rr : rr + D, jt * P : (jt + 1) * P],
                                qTh[:, bi * IB + 256 : (bi + 1) * IB],
                                start=True, stop=False,
                            )
                            nc.tensor.matmul(
                                sp[:, 0:P], trin_sb[:], identb[:],
                                start=False, stop=True,
                            )
                            nc.tensor.matmul(
                                sp[:, 256:384],
                                kT2[rr : rr + D, (jt + 1) * P : (jt + 2) * P],
                                qTh[:, bi * IB + 384 : (bi + 1) * IB],
                                start=True, stop=False,
                            )
                            nc.tensor.matmul(
                                sp[:, 256:384], trin_sb[:], identb[:],
                                start=False, stop=True,
                            )
                            nc.scalar.activation(
                                ex[:], sp[:], AF.Exp, scale=SC, bias=zero_sb[:],
                            )
                            pending.append((jt, ex[:, 0:256], 2, 2))
                            pending.append((jt + 1, ex[:, 256:384], 3, 3))
                            jt += 2
                        else:
                            sp = pssc.tile([P, 512], f32, tag="sc", name="sp")
                            ex = exps.tile([P, 512], bf16, tag="ex", name="ex")
                            lo, dg = score_mm(sp, jt)
                            nc.scalar.activation(
                                ex[:, lo:512], sp[:, lo:512], AF.Exp,
                                scale=SC, bias=zero_sb[:],
                            )
                            pending.append((jt, ex, dg, 0))
                            jt += 1
                        flush(3)
                    flush(0)
                    # the last head's hook (next group's phase1b) fires
                    # before its normalize tail so the qT/kT2 copies get
                    # ahead of the tail ops in the in-order DVE queue: the
                    # next block's first scores depend on them
                    if after_head is not None:
                        after_head(h)
                        fired_last = True
                    else:
                        fired_last = False
                    # normalize in [query, d] orientation: per-partition
                    # denominator scalars.  The PE transposes into the yT
                    # residents are deferred to the end of the bi block so the
                    # in-order PE queue never parks on a transpose whose yn
                    # input is still deep in the DVE queue (that would starve
                    # the score stream and the exp pipeline behind it).
                    rec = small.tile([P, GRP], f32, tag="rec")
                    nc.vector.reciprocal_approx_fast(rec[:], yp[:, :, D])
                    # hybrid normalize: one DVE bulk copy out of PSUM, then
                    # the per-qc scalar multiplies on the idle Pool engine
                    yn = ynp.tile([P, GRP, D], bf16, tag="yn")
                    if bi == NBI - 1 and h == HQ - 1:
                        # drain tail: direct DVE normalize, shortest chain
                        for qc in range(GRP):
                            nc.vector.tensor_scalar(
                                yn[:, qc, :], yp[:, qc, 0:D], rec[:, qc : qc + 1],
                                None, op0=mybir.AluOpType.mult,
                            )
                    else:
                        # hybrid: one DVE bulk copy out of PSUM, then the
                        # per-qc scalar multiplies on the idle Pool engine
                        yc = ynp.tile([P, GRP, D], bf16, tag="yc")
                        nc.vector.tensor_copy(yc[:], yp[:, :, 0:D])
                        for qc in range(GRP):
                            nc.gpsimd.tensor_scalar(
                                yn[:, qc, :], yc[:, qc, :], rec[:, qc : qc + 1],
                                None, op0=mybir.AluOpType.mult,
                            )
                    yns.append((h, yn))
                    if after_head is not None and not fired_last:
                        after_head(h)
                for h, yn in yns:
                    stg = psy.tile([D, IB], bf16, tag="y")
                    for qc in range(GRP):
                        nc.tensor.transpose(
                            stg[:, qc * P : (qc + 1) * P], yn[:, qc, :], identb[:]
                        )
                    ytp = yT1 if h < 2 else yT2
                    row = D * (h % 2)
                    nc.vector.tensor_copy(
                        ytp[row : row + D, bi * IB : (bi + 1) * IB], stg[:]
                    )

            def norm3w(bi, split_copies=False):
                # in the drain tail the score pool is free: 4 po slots keep
                # the Wo stream, readout copies and out-DMAs fully pipelined
                for tl in range(GRP):
                    tc_ = bi * GRP + tl
                    ob = outsb.tile([P, C], bf16, tag="ob")
                    for cb in range(2):
                        if split_copies:
                            po = pssc.tile([P, 512], f32, tag="sc")
                        else:
                            po = psmm.tile([P, 512], f32, tag="mm")
                        nc.tensor.matmul(
                            po[:],
                            yT1[:, tc_ * P : (tc_ + 1) * P],
                            wo1_sb[:, cb * 512 : (cb + 1) * 512],
                            start=True,
                            stop=False,
                        )
                        nc.tensor.matmul(
                            po[:],
                            yT2[:, tc_ * P : (tc_ + 1) * P],
                            wo2_sb[:, cb * 512 : (cb + 1) * 512],
                            start=False,
                            stop=True,
                        )
                        # in the drain tail ACT is idle: alternate the PSUM
                        # readout between DVE and ACT so po slots recycle 2x
                        # faster
                        if split_copies and cb == 1:
                            nc.scalar.copy(ob[:, cb * 512 : (cb + 1) * 512], po[:])
                        else:
                            nc.vector.tensor_copy(
                                ob[:, cb * 512 : (cb + 1) * 512], po[:]
                            )
                    nc.sync.dma_start(out[tc_ * P : (tc_ + 1) * P, :], ob[:])

            # group-level software pipeline: next group's projections are
            # emitted before the previous group's Wo so the PE has ready work
            # while the per-head normalize chains resolve.
            xts = {0: xt0, 1: xt1}
            qkns = {}
            phase1(0, xts[0], halves=2)
            phase1b(0)

            def hook0(h):
                # group 1's projections split across the thin bi-0 heads so
                # the 5us proj block doesn't starve the exp stream
                if h == 0:
                    phase1_proj(1, xts[1], [0, 1])
                    xts[2] = load_x(2)
                elif h == 1:
                    phase1_proj(1, xts[1], [2, 3])
                elif h == 2:
                    phase1(1, xts[1], proj=False)
                elif h == 3:
                    phase1b(1)

            phase2(0, after_head=hook0)
            for bi in range(1, NBI):
                # the next group's projections, transposes and bi-1's
                # Wo/writeout are deferred into phase2 via the per-head hook
                # so they do not sit ahead of the score/exp stream in the
                # in-order queues, and so the transposes finish well before
                # the group boundary
                def hook(h, bi=bi):
                    if h == 0:
                        if bi + 1 < NBI:
                            if bi == 1:
                                # phase2(1) is still thin: split group 2's
                                # projection block across two head hooks
                                phase1_proj(2, xts[2], [0, 1])
                            else:
                                phase1(bi + 1, xts[bi + 1])
                            if bi + 2 < NBI:
                                xts[bi + 2] = load_x(bi + 2)
                    elif h == 1:
                        if bi == 1:
                            phase1_proj(2, xts[2], [2, 3])
                            phase1(2, xts[2], proj=False)
                        norm3w(bi - 1)
                    elif h == 3 and bi + 1 < NBI:
                        phase1b(bi + 1)

                phase2(bi, after_head=hook)
                if bi == NBI - 1:
                    norm3w(bi, split_copies=True)
    nc.compile()
    return nc


def make_core_inputs(x, ve, cos, sin, Wq, Wk, Wv, Wo, Wg):
    """Slice full inputs into the 8 per-core input maps (b-major, then group)."""
    import ml_dtypes

    bf = ml_dtypes.bfloat16
    # device layout [P, NT*32]: row p holds cos[n*128+p, :] for n in 0..NT
    cosf = np.ascontiguousarray(
        cos[0, :, 0, :].reshape(NT, P, 32).transpose(1, 0, 2).reshape(P, NT * 32)
    ).astype(bf)
    sinf = np.ascontiguousarray(
        sin[0, :, 0, :].reshape(NT, P, 32).transpose(1, 0, 2).reshape(P, NT * 32)
    ).astype(bf)
    # trin[c, k] = -3e4 where key k > query c (strict upper): additive mask
    # accumulated into the diagonal score band via trin^T (identity moving).
    trin = np.where(
        np.arange(P)[None, :] > np.arange(P)[:, None], -30000.0, 0.0
    ).astype(bf)
    in_maps = []
    for c in range(8):
        b, g = c // N_KV_HEAD, c % N_KV_HEAD
        xTc = np.ascontiguousarray(x[b].T).astype(bf)  # [C, T]
        wq = Wq[g * 256 : (g + 1) * 256, :]           # [256, C]
        wk = Wk[g * D : (g + 1) * D, :]               # [64, C]
        wv = Wv[g * D : (g + 1) * D, :]
        wg_col = np.zeros((C, 1), np.float32)
        wg_col[:12, 0] = Wg[g]
        wrc = np.concatenate(
            [wq.T, wk.T, wv.T, wg_col, np.zeros((C, 1), np.float32)], axis=1
        ).astype(bf)                                  # [C, 386]
        ve3 = np.ascontiguousarray(
            (3.0 * ve[b, :, g * D : (g + 1) * D])
            .reshape(NT, P, D).transpose(1, 0, 2).reshape(P, NT * D)
        ).astype(bf)                                  # [P, NT*64]
        woTc = np.ascontiguousarray(
            Wo[:, g * 256 : (g + 1) * 256].T
        ).astype(bf)                                  # [256, C]
        in_maps.append(
            {
                "xT": xTc,
                "wr": np.ascontiguousarray(wrc),
                "cosd": cosf,
                "sind": sinf,
                "ve3": ve3,
                "woT": woTc,
                "trind": trin,
            }
        )
    return in_maps


_PROGRAM = None


def kernel(x, ve, cos, sin, Wq, Wk, Wv, Wo, Wg, _trace=False):
    from concourse.bass_utils import run_bass_kernel_spmd

    # coerce to host fp32 ndarrays up front (harness may pass jax arrays)
    x, ve, cos, sin, Wq, Wk, Wv, Wo, Wg = (
        np.asarray(a, dtype=np.float32)
        for a in (x, ve, cos, sin, Wq, Wk, Wv, Wo, Wg)
    )
    global _PROGRAM
    if _PROGRAM is None:
        _PROGRAM = build_program()
    nc = _PROGRAM
    in_maps = make_core_inputs(x, ve, cos, sin, Wq, Wk, Wv, Wo, Wg)
    res = run_bass_kernel_spmd(nc, in_maps, list(range(8)), trace=_trace)
    outs = [r["out"] for r in res.results]
    full = np.zeros((B, T, C), np.float32)
    for c in range(8):
        full[c // N_KV_HEAD] += np.asarray(outs[c], dtype=np.float32)
    if _trace:
        kernel.last_results = res
    return full

